# revision 1
# baseline (speedup 1.0000x reference)
"""AutoCorrelation multi-head attention (Autoformer-style) on 8 TRN2 NeuronCores.

Shapes (hardcoded): B=4, L=4096, DM=512, H=8, Dk=64, k=16.

Sharding: 8 cores = 4 batches x 2 head-groups (4 heads each).
Device graph A (per core): Q/K/V projections for its head group as dense
matmuls (contraction DM=512 on partitions, host passes x^T and W^T slices).
Host: FFT cross-correlation, top-k(16), softmax, rolled gather of V.
Device graph B (per core): output projection of a (2048, 512) row chunk.
Host adds biases (pure broadcast adds) and assembles the full output.
"""

import os
import sys
import math

for _p in ("/opt/trn_rl_repo",):
    if os.path.isdir(_p) and _p not in sys.path:
        sys.path.insert(0, _p)

import numpy as np

import concourse.bass as bass
import concourse.mybir as mybir
import concourse.tile as tile
from concourse.bass_utils import run_bass_kernel_spmd

B, L, DM, H, DK = 4, 4096, 512, 8, 64
KTOP = 16
N_CORES = 8
F32 = mybir.dt.float32
F32R = mybir.dt.float32r

_GRAPHS = {}


def _matmul_graph(n_dim, in_name, w_name, w_cols, in_dt=F32R, out_dt=F32):
    """out[w_cols, n_dim] = w.T @ data, data [DM=512, n_dim], w [DM, w_cols].

    Raw-bass pipelined: sync engine DMAs in/out, PE accumulates over 4 k-tiles
    of 128 into PSUM, DVE evicts PSUM->SBUF. One explicit semaphore wait per
    instruction (walrus limits sync-wait slots per instruction).
    """
    nc = bass.Bass()
    data = nc.dram_tensor(in_name, [DM, n_dim], in_dt, kind="ExternalInput")
    w = nc.dram_tensor(w_name, [DM, w_cols], in_dt, kind="ExternalInput")
    out = nc.dram_tensor("out", [w_cols, n_dim], out_dt, kind="ExternalOutput")

    n_chunks = n_dim // 512
    m_tiles = w_cols // 128
    n_groups = m_tiles * n_chunks
    NPS = 8  # psum buffers (all 8 banks)
    NEV = 6  # sbuf eviction buffers

    with (
        nc.sbuf_tensor([128, 4 * n_dim], in_dt) as x_sb,
        nc.sbuf_tensor([128, 4 * w_cols], in_dt) as w_sb,
        nc.sbuf_tensor([128, NEV * 512], out_dt) as ev_sb,
        nc.psum_tensor([128, NPS * 512], F32) as ps,
        nc.semaphore() as dma_sem,
        nc.semaphore() as pe_sem,
        nc.semaphore() as dve_sem,
        nc.semaphore() as odma_sem,
        nc.Block() as block,
    ):
        n_in_dmas = 4 + 4 * n_chunks

        @block.sync
        def _(sync):
            for k in range(4):
                sync.dma_start(
                    w_sb[:, w_cols * k : w_cols * (k + 1)],
                    w[128 * k : 128 * (k + 1), :],
                ).then_inc(dma_sem, 16)
            for ntc in range(n_chunks):
                for k in range(4):
                    sync.dma_start(
                        x_sb[:, n_dim * k + 512 * ntc : n_dim * k + 512 * (ntc + 1)],
                        data[128 * k : 128 * (k + 1), 512 * ntc : 512 * (ntc + 1)],
                    ).then_inc(dma_sem, 16)
            for g in range(n_groups):
                mt, ntc = divmod(g, n_chunks)
                sync.wait_ge(dve_sem, g + 1)
                sync.dma_start(
                    out[128 * mt : 128 * (mt + 1), 512 * ntc : 512 * (ntc + 1)],
                    ev_sb[:, 512 * (g % NEV) : 512 * (g % NEV + 1)],
                ).then_inc(odma_sem, 16)

        @block.tensor
        def _(tensor):
            dma_gate = 0
            for g in range(n_groups):
                mt, ntc = divmod(g, n_chunks)
                # inputs needed: 4 w DMAs + x chunks for columns <= ntc
                need = 16 * (4 + 4 * (ntc + 1))
                if need > dma_gate:
                    tensor.wait_ge(dma_sem, need)
                    dma_gate = need
                if g >= NPS:
                    tensor.wait_ge(dve_sem, g - NPS + 1)
                pslice = ps[:, 512 * (g % NPS) : 512 * (g % NPS + 1)]
                for kt in range(4):
                    mm = nc.tensor.matmul(
                        pslice,
                        w_sb[:, w_cols * kt + 128 * mt : w_cols * kt + 128 * (mt + 1)],
                        x_sb[:, n_dim * kt + 512 * ntc : n_dim * kt + 512 * (ntc + 1)],
                        start=(kt == 0),
                        stop=(kt == 3),
                    )
                    if kt == 3:
                        mm.then_inc(pe_sem, 1)

        @block.vector
        def _(vector):
            for g in range(n_groups):
                vector.wait_ge(pe_sem, g + 1)
                if g >= NEV:
                    vector.wait_ge(odma_sem, 16 * (g - NEV + 1))
                nc.vector.tensor_copy(
                    ev_sb[:, 512 * (g % NEV) : 512 * (g % NEV + 1)],
                    ps[:, 512 * (g % NPS) : 512 * (g % NPS + 1)],
                ).then_inc(dve_sem, 1)

    return nc


def _get_graphs():
    if not _GRAPHS:
        _GRAPHS["proj"] = _matmul_graph(L, "xt", "w", 768)
        _GRAPHS["outproj"] = _matmul_graph(2048, "ct", "wot", DM)
        # Discard-first warm-up: the very first execution of a freshly
        # compiled graph has been observed (rarely) to race on one core;
        # warm executions are deterministic. Run each graph once on zeros.
        z = np.zeros((DM, L), np.float32)
        zw = np.zeros((DM, 768), np.float32)
        run_bass_kernel_spmd(
            _GRAPHS["proj"],
            [{"xt": z, "w": zw} for _ in range(N_CORES)],
            core_ids=list(range(N_CORES)),
        )
        zc = np.zeros((DM, 2048), np.float32)
        zo = np.zeros((DM, DM), np.float32)
        run_bass_kernel_spmd(
            _GRAPHS["outproj"],
            [{"ct": zc, "wot": zo} for _ in range(N_CORES)],
            core_ids=list(range(N_CORES)),
        )
    return _GRAPHS


LAST_EXEC_NS = [None, None]


def kernel(x, Wq, bq, Wk, bk, Wv, bv, Wo, bo):
    x = np.asarray(x, np.float32)
    Wq, bq = np.asarray(Wq, np.float32), np.asarray(bq, np.float32)
    Wk, bk = np.asarray(Wk, np.float32), np.asarray(bk, np.float32)
    Wv, bv = np.asarray(Wv, np.float32), np.asarray(bv, np.float32)
    Wo, bo = np.asarray(Wo, np.float32), np.asarray(bo, np.float32)
    g = _get_graphs()

    # ---- device graph A: QKV projections ----
    wqT, wkT, wvT = Wq.T.copy(), Wk.T.copy(), Wv.T.copy()  # (DM_in, DM_out)
    in_maps = []
    for core in range(N_CORES):
        b, half = core // 2, core % 2
        dsl = slice(half * 256, (half + 1) * 256)
        w_core = np.ascontiguousarray(
            np.concatenate([wqT[:, dsl], wkT[:, dsl], wvT[:, dsl]], axis=1),
            np.float32,
        )
        xt_core = np.ascontiguousarray(x[b].T, np.float32)
        in_maps.append({"xt": xt_core, "w": w_core})
    resA = run_bass_kernel_spmd(g["proj"], in_maps, core_ids=list(range(N_CORES)))
    for _ in range(3):
        resA2 = run_bass_kernel_spmd(
            g["proj"], in_maps, core_ids=list(range(N_CORES))
        )
        if all(
            np.array_equal(resA.results[c]["out"], resA2.results[c]["out"])
            for c in range(N_CORES)
        ):
            break
        resA = resA2
    LAST_EXEC_NS[0] = resA.exec_time_ns

    # Assemble Q, K, V as (B, H, L, Dk), adding biases on host
    Q = np.empty((B, H, L, DK), np.float32)
    K = np.empty((B, H, L, DK), np.float32)
    V = np.empty((B, H, L, DK), np.float32)
    for core in range(N_CORES):
        b, half = core // 2, core % 2
        o = resA.results[core]["out"]  # (768, L)
        for j, (dst, bias) in enumerate(((Q, bq), (K, bk), (V, bv))):
            blk = o[256 * j : 256 * (j + 1)]  # (256, L) rows = local d
            for hl in range(4):
                h = half * 4 + hl
                dst[b, h] = (
                    blk[64 * hl : 64 * (hl + 1)].T
                    + bias[256 * half + 64 * hl : 256 * half + 64 * (hl + 1)]
                )

    # ---- host: FFT autocorrelation + top-k + rolled gather ----
    try:
        from scipy import fft as sfft

        def _rfft(a):
            return sfft.rfft(a, axis=2, workers=8)

        def _irfft(a):
            return sfft.irfft(a, n=L, axis=2, workers=8)

    except Exception:

        def _rfft(a):
            return np.fft.rfft(a, axis=2)

        def _irfft(a):
            return np.fft.irfft(a, n=L, axis=2)

    qf = _rfft(Q)
    kf = _rfft(K)
    S = np.einsum("bhfd,bhfd->bhf", qf, np.conj(kf))  # (B, H, Lf)
    corr_mean = _irfft(S) / DK  # (B, H, L)

    k = min(int(2 * math.log(L)), L)  # 16
    order = np.argsort(-corr_mean, axis=-1, kind="stable")
    delays = order[..., :k]  # (B, H, k)
    wvals = np.take_along_axis(corr_mean, delays, axis=-1)
    wvals = wvals - wvals.max(axis=-1, keepdims=True)
    wexp = np.exp(wvals)
    wsm = (wexp / wexp.sum(axis=-1, keepdims=True)).astype(np.float32)

    ctx = np.empty((B, H, L, DK), np.float32)
    t_arange = np.arange(L)
    for b in range(B):
        for h in range(H):
            idx = (t_arange[:, None] - delays[b, h][None, :]) % L  # (L, k)
            ctx[b, h] = np.einsum(
                "lkd,k->ld", V[b, h][idx], wsm[b, h], optimize=True
            )
    ctx_flat = ctx.transpose(0, 2, 1, 3).reshape(B, L, DM)

    # ---- device graph B: output projection ----
    woT = np.ascontiguousarray(Wo.T, np.float32)
    in_maps_b = []
    for core in range(N_CORES):
        b, half = core // 2, core % 2
        chunk = ctx_flat[b, half * 2048 : (half + 1) * 2048]  # (2048, DM)
        in_maps_b.append(
            {"ct": np.ascontiguousarray(chunk.T, np.float32), "wot": woT}
        )
    # Rare intermittent single-group corruption has been observed on this
    # graph's executions; corrupted runs differ from clean ones (and from each
    # other), so accept only a result reproduced by two runs.
    resB = run_bass_kernel_spmd(g["outproj"], in_maps_b, core_ids=list(range(N_CORES)))
    for _ in range(3):
        resB2 = run_bass_kernel_spmd(
            g["outproj"], in_maps_b, core_ids=list(range(N_CORES))
        )
        if all(
            np.array_equal(resB.results[c]["out"], resB2.results[c]["out"])
            for c in range(N_CORES)
        ):
            break
        resB = resB2
    LAST_EXEC_NS[1] = resB.exec_time_ns

    out = np.empty((B, L, DM), np.float32)
    for core in range(N_CORES):
        b, half = core // 2, core % 2
        out[b, half * 2048 : (half + 1) * 2048] = resB.results[core]["out"].T
    out += bo.astype(np.float32)
    return out



# revision 7
# speedup vs baseline: 6.8155x; 6.8155x over previous
"""AutoCorrelation multi-head attention (Autoformer-style) on 8 TRN2 NeuronCores.

Shapes (hardcoded): B=4, L=4096, DM=512, H=8, Dk=64, k=16.

Sharding: 8 cores = 4 batches x 2 head-groups (4 heads each) for the QKV
projections; 4 batches x 2 token-halves for the output projection.

The axon tunnel to the devices moves ~33MB/s, so the design minimizes
host<->device bytes: all tunnel transfers are bf16 (tolerance is 2e-2;
bf16 keeps us ~5e-3), graphs are built with the Tile framework (auto
synchronization - no manual-semaphore races, so a single execution per
graph is trusted), and only two device calls are made per kernel() run.

Device graph A (per core): Q/K/V projections for its head group as dense
matmuls (contraction DM=512 on partitions; host passes x^T and W^T slices).
Host: FFT cross-correlation, top-k(16), softmax, rolled gather of V.
Device graph B (per core): output projection of a 2048-token chunk.
Host adds biases (pure broadcast adds) and assembles the full output.
"""

import os
import sys
import math

for _p in ("/opt/trn_rl_repo",):
    if os.path.isdir(_p) and _p not in sys.path:
        sys.path.insert(0, _p)

import numpy as np
import ml_dtypes

import concourse.bass as bass
import concourse.mybir as mybir
import concourse.tile as tile
from concourse.bass_utils import run_bass_kernel_spmd

B, L, DM, H, DK = 4, 4096, 512, 8, 64
KTOP = 16
N_CORES = 8
F32 = mybir.dt.float32
BF16 = mybir.dt.bfloat16
NPBF16 = ml_dtypes.bfloat16

_GRAPHS = {}


def _mm_graph(n_dim, w_cols):
    """out[w_cols, n_dim] = w.T @ data  (bf16 in, f32 psum accum, bf16 out).

    data [DM=512, n_dim], w [DM, w_cols].

    Raw bass (this walrus build allows only ONE sync wait per instruction,
    so Tile's multi-wait tail drain cannot compile; multi-waits here are
    standalone wait_ge instructions). Race-free by construction: exactly one
    DMA per dram tensor, each completing on its own semaphore, so every
    wait threshold identifies a unique DMA's completion (the old graph
    counted completions of many DMAs on one semaphore, which assumed
    cross-queue in-order completion - the source of the rare corruption).
    pe_sem/dve_sem count single in-order engine queues, which is exact.
    """
    nc = bass.Bass()
    data = nc.dram_tensor("data", [DM, n_dim], BF16, kind="ExternalInput")
    w = nc.dram_tensor("w", [DM, w_cols], BF16, kind="ExternalInput")
    out = nc.dram_tensor("out", [w_cols, n_dim], BF16, kind="ExternalOutput")

    kt_n = DM // 128  # 4 contraction tiles
    mt_n = w_cols // 128  # output-row tiles
    nc_n = n_dim // 512  # output-col chunks
    n_groups = mt_n * nc_n
    NPS = 8  # psum banks cycled

    with (
        nc.sbuf_tensor([128, kt_n, n_dim], BF16) as xs,
        nc.sbuf_tensor([128, kt_n, w_cols], BF16) as wt,
        nc.sbuf_tensor([128, mt_n, nc_n, 512], BF16) as ev,
        nc.psum_tensor([128, NPS, 512], F32) as ps,
        nc.semaphore() as w_sem,
        nc.semaphore() as x_sem,
        nc.semaphore() as pe_sem,
        nc.semaphore() as dve_sem,
        nc.semaphore() as odma_sem,
        nc.Block() as block,
    ):

        @block.sync
        def _(sync):
            sync.dma_start(wt[:, :, :], w.rearrange("(kt p) m -> p kt m", p=128)).then_inc(
                w_sem, 16
            )
            sync.dma_start(xs[:, :, :], data.rearrange("(kt p) n -> p kt n", p=128)).then_inc(
                x_sem, 16
            )
            sync.wait_ge(dve_sem, n_groups)
            sync.dma_start(
                out.rearrange("(mt p) (ntc c) -> p mt ntc c", p=128, c=512),
                ev[:, :, :, :],
            ).then_inc(odma_sem, 16)

        @block.tensor
        def _(tensor):
            tensor.wait_ge(w_sem, 16)
            tensor.wait_ge(x_sem, 16)
            for g in range(n_groups):
                mt, ntc = divmod(g, nc_n)
                if g >= NPS:
                    tensor.wait_ge(dve_sem, g - NPS + 1)
                for kt in range(kt_n):
                    mm = nc.tensor.matmul(
                        ps[:, g % NPS, :],
                        wt[:, kt, 128 * mt : 128 * (mt + 1)],
                        xs[:, kt, 512 * ntc : 512 * (ntc + 1)],
                        start=(kt == 0),
                        stop=(kt == kt_n - 1),
                    )
                    if kt == kt_n - 1:
                        mm.then_inc(pe_sem, 1)

        @block.vector
        def _(vector):
            for g in range(n_groups):
                mt, ntc = divmod(g, nc_n)
                vector.wait_ge(pe_sem, g + 1)
                nc.vector.tensor_copy(ev[:, mt, ntc, :], ps[:, g % NPS, :]).then_inc(
                    dve_sem, 1
                )

    return nc


def _get_graphs():
    if not _GRAPHS:
        _GRAPHS["proj"] = _mm_graph(L, 768)
        _GRAPHS["outproj"] = _mm_graph(2048, DM)
    return _GRAPHS


LAST_EXEC_NS = [None, None]


def _bf16(a):
    return np.ascontiguousarray(a, dtype=NPBF16)


def kernel(x, Wq, bq, Wk, bk, Wv, bv, Wo, bo):
    x = np.asarray(x, np.float32)
    Wq, bq = np.asarray(Wq, np.float32), np.asarray(bq, np.float32)
    Wk, bk = np.asarray(Wk, np.float32), np.asarray(bk, np.float32)
    Wv, bv = np.asarray(Wv, np.float32), np.asarray(bv, np.float32)
    Wo, bo = np.asarray(Wo, np.float32), np.asarray(bo, np.float32)
    g = _get_graphs()

    # ---- device graph A: QKV projections (channel-major layouts) ----
    wqT, wkT, wvT = Wq.T, Wk.T, Wv.T  # (DM_in, DM_out)
    in_maps = []
    for core in range(N_CORES):
        b, half = core // 2, core % 2
        dsl = slice(half * 256, (half + 1) * 256)
        w_core = _bf16(np.concatenate([wqT[:, dsl], wkT[:, dsl], wvT[:, dsl]], axis=1))
        in_maps.append({"data": _bf16(x[b].T), "w": w_core})
    resA = run_bass_kernel_spmd(g["proj"], in_maps, core_ids=list(range(N_CORES)))
    LAST_EXEC_NS[0] = resA.exec_time_ns

    # Assemble channel-major QT/KT/VT (B, DM, L), adding biases on host.
    QT = np.empty((B, DM, L), np.float32)
    KT = np.empty((B, DM, L), np.float32)
    VT = np.empty((B, DM, L), np.float32)
    for core in range(N_CORES):
        b, half = core // 2, core % 2
        o = np.asarray(resA.results[core]["out"]).astype(np.float32)  # (768, L)
        dsl = slice(half * 256, (half + 1) * 256)
        QT[b, dsl] = o[0:256] + bq[dsl][:, None]
        KT[b, dsl] = o[256:512] + bk[dsl][:, None]
        VT[b, dsl] = o[512:768] + bv[dsl][:, None]

    # ---- host: FFT autocorrelation + top-k + rolled gather ----
    try:
        from scipy import fft as sfft

        def _rfft(a):
            return sfft.rfft(a, axis=-1, workers=8)

        def _irfft(a):
            return sfft.irfft(a, n=L, axis=-1, workers=8)

    except Exception:

        def _rfft(a):
            return np.fft.rfft(a, axis=-1)

        def _irfft(a):
            return np.fft.irfft(a, n=L, axis=-1)

    qf = _rfft(QT)  # (B, DM, Lf)
    kf = _rfft(KT)
    prod = qf * np.conj(kf)
    S = prod.reshape(B, H, DK, -1).sum(axis=2)  # (B, H, Lf)
    corr_mean = _irfft(S) / DK  # (B, H, L)

    k = min(int(2 * math.log(L)), L)  # 16
    order = np.argsort(-corr_mean, axis=-1, kind="stable")
    delays = order[..., :k]  # (B, H, k)
    wvals = np.take_along_axis(corr_mean, delays, axis=-1)
    wvals = wvals - wvals.max(axis=-1, keepdims=True)
    wexp = np.exp(wvals)
    wsm = (wexp / wexp.sum(axis=-1, keepdims=True)).astype(np.float32)

    # ctxT[b, c, t] = sum_j w_j * VT[b, c, (t - delay_j) % L]
    ctxT = np.zeros((B, DM, L), np.float32)
    for b in range(B):
        for h in range(H):
            csl = slice(64 * h, 64 * (h + 1))
            acc = ctxT[b, csl]
            for j in range(k):
                d = int(delays[b, h, j])
                wj = wsm[b, h, j]
                rolled = np.concatenate(
                    [VT[b, csl, L - d :], VT[b, csl, : L - d]], axis=1
                )
                acc += wj * rolled
    # ---- device graph B: output projection ----
    woT = _bf16(Wo.T)
    in_maps_b = []
    for core in range(N_CORES):
        b, half = core // 2, core % 2
        chunk = ctxT[b][:, half * 2048 : (half + 1) * 2048]  # (DM, 2048)
        in_maps_b.append({"data": _bf16(chunk), "w": woT})
    resB = run_bass_kernel_spmd(g["outproj"], in_maps_b, core_ids=list(range(N_CORES)))
    LAST_EXEC_NS[1] = resB.exec_time_ns

    out = np.empty((B, L, DM), np.float32)
    for core in range(N_CORES):
        b, half = core // 2, core % 2
        ob = np.asarray(resB.results[core]["out"]).astype(np.float32)  # (512, 2048)
        out[b, half * 2048 : (half + 1) * 2048] = ob.T
    out += bo.astype(np.float32)
    return out


# revision 10
# speedup vs baseline: 8.8065x; 1.2921x over previous
"""AutoCorrelation multi-head attention (Autoformer-style) on 8 TRN2 NeuronCores.

Shapes (hardcoded): B=4, L=4096, DM=512, H=8, Dk=64, k=16.

Sharding: 8 cores = 4 batches x 2 head-groups (4 heads each) for the QKV
projections; 4 batches x 2 token-halves for the output projection.

The axon tunnel to the devices moves ~33MB/s, so the design minimizes
host<->device bytes: all tunnel transfers are bf16 (tolerance is 2e-2;
bf16 keeps us ~5e-3), graphs are built with the Tile framework (auto
synchronization - no manual-semaphore races, so a single execution per
graph is trusted), and only two device calls are made per kernel() run.

Device graph A (per core): Q/K/V projections for its head group as dense
matmuls (contraction DM=512 on partitions; host passes x^T and W^T slices).
Host: FFT cross-correlation, top-k(16), softmax, rolled gather of V.
Device graph B (per core): output projection of a 2048-token chunk.
Host adds biases (pure broadcast adds) and assembles the full output.
"""

import os
import sys
import math

for _p in ("/opt/trn_rl_repo",):
    if os.path.isdir(_p) and _p not in sys.path:
        sys.path.insert(0, _p)

import numpy as np
import ml_dtypes

import concourse.bass as bass
import concourse.mybir as mybir
import concourse.tile as tile
from concourse.bass_utils import run_bass_kernel_spmd

B, L, DM, H, DK = 4, 4096, 512, 8, 64
KTOP = 16
N_CORES = 8
F32 = mybir.dt.float32
BF16 = mybir.dt.bfloat16
NPBF16 = ml_dtypes.bfloat16

_GRAPHS = {}


def _mm_graph(n_dim, w_cols):
    """out[w_cols, n_dim] = w.T @ data  (bf16 in, f32 psum accum, bf16 out).

    data [DM=512, n_dim], w [DM, w_cols].

    Raw bass (this walrus build allows only ONE sync wait per instruction,
    so Tile's multi-wait tail drain cannot compile; multi-waits here are
    standalone wait_ge instructions). Race-free by construction: exactly one
    DMA per dram tensor, each completing on its own semaphore, so every
    wait threshold identifies a unique DMA's completion (the old graph
    counted completions of many DMAs on one semaphore, which assumed
    cross-queue in-order completion - the source of the rare corruption).
    pe_sem/dve_sem count single in-order engine queues, which is exact.
    """
    nc = bass.Bass()
    data = nc.dram_tensor("data", [DM, n_dim], BF16, kind="ExternalInput")
    w = nc.dram_tensor("w", [DM, w_cols], BF16, kind="ExternalInput")
    out = nc.dram_tensor("out", [w_cols, n_dim], BF16, kind="ExternalOutput")

    kt_n = DM // 128  # 4 contraction tiles
    mt_n = w_cols // 128  # output-row tiles
    nc_n = n_dim // 512  # output-col chunks
    n_groups = mt_n * nc_n
    NPS = 8  # psum banks cycled

    with (
        nc.sbuf_tensor([128, kt_n, n_dim], BF16) as xs,
        nc.sbuf_tensor([128, kt_n, w_cols], BF16) as wt,
        nc.sbuf_tensor([128, mt_n, nc_n, 512], BF16) as ev,
        nc.psum_tensor([128, NPS, 512], F32) as ps,
        nc.semaphore() as w_sem,
        nc.semaphore() as x_sem,
        nc.semaphore() as pe_sem,
        nc.semaphore() as dve_sem,
        nc.semaphore() as odma_sem,
        nc.Block() as block,
    ):

        @block.sync
        def _(sync):
            sync.dma_start(wt[:, :, :], w.rearrange("(kt p) m -> p kt m", p=128)).then_inc(
                w_sem, 16
            )
            sync.dma_start(xs[:, :, :], data.rearrange("(kt p) n -> p kt n", p=128)).then_inc(
                x_sem, 16
            )
            sync.wait_ge(dve_sem, n_groups)
            sync.dma_start(
                out.rearrange("(mt p) (ntc c) -> p mt ntc c", p=128, c=512),
                ev[:, :, :, :],
            ).then_inc(odma_sem, 16)

        @block.tensor
        def _(tensor):
            tensor.wait_ge(w_sem, 16)
            tensor.wait_ge(x_sem, 16)
            for g in range(n_groups):
                mt, ntc = divmod(g, nc_n)
                if g >= NPS:
                    tensor.wait_ge(dve_sem, g - NPS + 1)
                for kt in range(kt_n):
                    mm = nc.tensor.matmul(
                        ps[:, g % NPS, :],
                        wt[:, kt, 128 * mt : 128 * (mt + 1)],
                        xs[:, kt, 512 * ntc : 512 * (ntc + 1)],
                        start=(kt == 0),
                        stop=(kt == kt_n - 1),
                    )
                    if kt == kt_n - 1:
                        mm.then_inc(pe_sem, 1)

        @block.vector
        def _(vector):
            for g in range(n_groups):
                mt, ntc = divmod(g, nc_n)
                vector.wait_ge(pe_sem, g + 1)
                nc.vector.tensor_copy(ev[:, mt, ntc, :], ps[:, g % NPS, :]).then_inc(
                    dve_sem, 1
                )

    return nc


def _get_graphs():
    if not _GRAPHS:
        _GRAPHS["proj"] = _mm_graph(L, 1536)
        _GRAPHS["outproj"] = _mm_graph(L, DM)
    return _GRAPHS


LAST_EXEC_NS = [None, None]


def _bf16(a):
    return np.ascontiguousarray(a, dtype=NPBF16)


def kernel(x, Wq, bq, Wk, bk, Wv, bv, Wo, bo):
    x = np.asarray(x, np.float32)
    Wq, bq = np.asarray(Wq, np.float32), np.asarray(bq, np.float32)
    Wk, bk = np.asarray(Wk, np.float32), np.asarray(bk, np.float32)
    Wv, bv = np.asarray(Wv, np.float32), np.asarray(bv, np.float32)
    Wo, bo = np.asarray(Wo, np.float32), np.asarray(bo, np.float32)
    g = _get_graphs()

    # ---- device graph A: QKV projections, one core per batch ----
    # (b-sharding on 4 cores halves the x upload vs (b, head-half) on 8:
    # the axon tunnel is the bottleneck, not device compute.)
    w_all = _bf16(np.concatenate([Wq.T, Wk.T, Wv.T], axis=1))  # (DM, 1536)
    in_maps = [{"data": _bf16(x[b].T), "w": w_all} for b in range(B)]
    resA = run_bass_kernel_spmd(g["proj"], in_maps, core_ids=list(range(B)))
    LAST_EXEC_NS[0] = resA.exec_time_ns

    # Assemble channel-major QT/KT/VT (B, DM, L), adding biases on host.
    QT = np.empty((B, DM, L), np.float32)
    KT = np.empty((B, DM, L), np.float32)
    VT = np.empty((B, DM, L), np.float32)
    for b in range(B):
        o = np.asarray(resA.results[b]["out"]).astype(np.float32)  # (1536, L)
        QT[b] = o[0:512] + bq[:, None]
        KT[b] = o[512:1024] + bk[:, None]
        VT[b] = o[1024:1536] + bv[:, None]

    # ---- host: FFT autocorrelation + top-k + rolled gather ----
    try:
        from scipy import fft as sfft

        def _rfft(a):
            return sfft.rfft(a, axis=-1, workers=8)

        def _irfft(a):
            return sfft.irfft(a, n=L, axis=-1, workers=8)

    except Exception:

        def _rfft(a):
            return np.fft.rfft(a, axis=-1)

        def _irfft(a):
            return np.fft.irfft(a, n=L, axis=-1)

    qf = _rfft(QT)  # (B, DM, Lf)
    kf = _rfft(KT)
    prod = qf * np.conj(kf)
    S = prod.reshape(B, H, DK, -1).sum(axis=2)  # (B, H, Lf)
    corr_mean = _irfft(S) / DK  # (B, H, L)

    k = min(int(2 * math.log(L)), L)  # 16
    order = np.argsort(-corr_mean, axis=-1, kind="stable")
    delays = order[..., :k]  # (B, H, k)
    wvals = np.take_along_axis(corr_mean, delays, axis=-1)
    wvals = wvals - wvals.max(axis=-1, keepdims=True)
    wexp = np.exp(wvals)
    wsm = (wexp / wexp.sum(axis=-1, keepdims=True)).astype(np.float32)

    # ctxT[b, c, t] = sum_j w_j * VT[b, c, (t - delay_j) % L]
    ctxT = np.zeros((B, DM, L), np.float32)
    for b in range(B):
        for h in range(H):
            csl = slice(64 * h, 64 * (h + 1))
            acc = ctxT[b, csl]
            for j in range(k):
                d = int(delays[b, h, j])
                wj = wsm[b, h, j]
                rolled = np.concatenate(
                    [VT[b, csl, L - d :], VT[b, csl, : L - d]], axis=1
                )
                acc += wj * rolled
    # ---- device graph B: output projection, one core per batch ----
    woT = _bf16(Wo.T)
    in_maps_b = [{"data": _bf16(ctxT[b]), "w": woT} for b in range(B)]
    resB = run_bass_kernel_spmd(g["outproj"], in_maps_b, core_ids=list(range(B)))
    LAST_EXEC_NS[1] = resB.exec_time_ns

    out = np.empty((B, L, DM), np.float32)
    for b in range(B):
        ob = np.asarray(resB.results[b]["out"]).astype(np.float32)  # (512, L)
        out[b] = ob.T
    out += bo.astype(np.float32)
    return out


# revision 13
# speedup vs baseline: 11.0495x; 1.2547x over previous
"""AutoCorrelation multi-head attention (Autoformer-style) on 8 TRN2 NeuronCores.

Shapes (hardcoded): B=4, L=4096, DM=512, H=8, Dk=64, k=16.

Sharding: 8 cores = 4 batches x 2 head-groups (4 heads each) for the QKV
projections; 4 batches x 2 token-halves for the output projection.

The axon tunnel to the devices moves ~33MB/s, so the design minimizes
host<->device bytes: all tunnel transfers are bf16 (tolerance is 2e-2;
bf16 keeps us ~5e-3), graphs are built with the Tile framework (auto
synchronization - no manual-semaphore races, so a single execution per
graph is trusted), and only two device calls are made per kernel() run.

Device graph A (per core): Q/K/V projections for its head group as dense
matmuls (contraction DM=512 on partitions; host passes x^T and W^T slices).
Host: FFT cross-correlation, top-k(16), softmax, rolled gather of V.
Device graph B (per core): output projection of a 2048-token chunk.
Host adds biases (pure broadcast adds) and assembles the full output.
"""

import os
import sys
import math

for _p in ("/opt/trn_rl_repo",):
    if os.path.isdir(_p) and _p not in sys.path:
        sys.path.insert(0, _p)

import numpy as np
import ml_dtypes

import concourse.bass as bass
import concourse.mybir as mybir
from concourse.bass import AP
from concourse.bass_utils import run_bass_kernel_spmd

B, L, DM, H, DK = 4, 4096, 512, 8, 64
KTOP = 16
N_CORES = 8
F32 = mybir.dt.float32
BF16 = mybir.dt.bfloat16
NPBF16 = ml_dtypes.bfloat16

_GRAPHS = {}


def _mm_graph(n_dim, w_cols):
    """out[w_cols, n_dim] = w.T @ data  (bf16 in, f32 psum accum, bf16 out).

    data [DM=512, n_dim], w [DM, w_cols].

    Raw bass (this walrus build allows only ONE sync wait per instruction,
    so Tile's multi-wait tail drain cannot compile; multi-waits here are
    standalone wait_ge instructions). Race-free by construction: exactly one
    DMA per dram tensor, each completing on its own semaphore, so every
    wait threshold identifies a unique DMA's completion (the old graph
    counted completions of many DMAs on one semaphore, which assumed
    cross-queue in-order completion - the source of the rare corruption).
    pe_sem/dve_sem count single in-order engine queues, which is exact.
    """
    nc = bass.Bass()
    data = nc.dram_tensor("data", [DM, n_dim], BF16, kind="ExternalInput")
    w = nc.dram_tensor("w", [DM, w_cols], BF16, kind="ExternalInput")
    out = nc.dram_tensor("out", [w_cols, n_dim], BF16, kind="ExternalOutput")

    kt_n = DM // 128  # 4 contraction tiles
    mt_n = w_cols // 128  # output-row tiles
    nc_n = n_dim // 512  # output-col chunks
    n_groups = mt_n * nc_n
    NPS = 8  # psum banks cycled

    with (
        nc.sbuf_tensor([128, kt_n, n_dim], BF16) as xs,
        nc.sbuf_tensor([128, kt_n, w_cols], BF16) as wt,
        nc.sbuf_tensor([128, mt_n, nc_n, 512], BF16) as ev,
        nc.psum_tensor([128, NPS, 512], F32) as ps,
        nc.semaphore() as w_sem,
        nc.semaphore() as x_sem,
        nc.semaphore() as pe_sem,
        nc.semaphore() as dve_sem,
        nc.semaphore() as odma_sem,
        nc.Block() as block,
    ):

        @block.sync
        def _(sync):
            sync.dma_start(wt[:, :, :], w.rearrange("(kt p) m -> p kt m", p=128)).then_inc(
                w_sem, 16
            )
            sync.dma_start(xs[:, :, :], data.rearrange("(kt p) n -> p kt n", p=128)).then_inc(
                x_sem, 16
            )
            sync.wait_ge(dve_sem, n_groups)
            sync.dma_start(
                out.rearrange("(mt p) (ntc c) -> p mt ntc c", p=128, c=512),
                ev[:, :, :, :],
            ).then_inc(odma_sem, 16)
            sync.wait_ge(odma_sem, 16)

        @block.tensor
        def _(tensor):
            tensor.wait_ge(w_sem, 16)
            tensor.wait_ge(x_sem, 16)
            for g in range(n_groups):
                mt, ntc = divmod(g, nc_n)
                if g >= NPS:
                    tensor.wait_ge(dve_sem, g - NPS + 1)
                for kt in range(kt_n):
                    mm = nc.tensor.matmul(
                        ps[:, g % NPS, :],
                        wt[:, kt, 128 * mt : 128 * (mt + 1)],
                        xs[:, kt, 512 * ntc : 512 * (ntc + 1)],
                        start=(kt == 0),
                        stop=(kt == kt_n - 1),
                    )
                    if kt == kt_n - 1:
                        mm.then_inc(pe_sem, 1)

        @block.vector
        def _(vector):
            for g in range(n_groups):
                mt, ntc = divmod(g, nc_n)
                vector.wait_ge(pe_sem, g + 1)
                nc.vector.tensor_copy(ev[:, mt, ntc, :], ps[:, g % NPS, :]).then_inc(
                    dve_sem, 1
                )

    return nc



MULT = mybir.AluOpType.mult
ADD = mybir.AluOpType.add
SUB = mybir.AluOpType.subtract

ORDER = [("q", 0), ("k", 0), ("q", 1), ("k", 1), ("q", 2), ("k", 2), ("q", 3), ("k", 3)]
N_PROJ = 96  # proj psum groups; then 160 per (tensor, chunk): 128 S1 + 32 S2


def fft_consts():
    j = np.arange(64)
    wc = np.cos(2 * np.pi * np.outer(j, j) / 64)
    ws = np.sin(2 * np.pi * np.outer(j, j) / 64)
    wfft = np.concatenate([wc, -ws, ws], axis=1)  # [Wc | Wms | Ws] (64, 192)
    tc = np.cos(2 * np.pi * np.outer(j, j) / L)  # [b, u]
    tms = -np.sin(2 * np.pi * np.outer(j, j) / L)
    twid = np.zeros((128, 128), np.float32)
    twid[0:64, 0:64] = tc
    twid[0:64, 64:128] = tms
    return (
        np.ascontiguousarray(wfft, NPBF16),
        np.ascontiguousarray(twid, NPBF16),
    )


def _bcast_j(ap64):
    """[64(b), 64(u)] -> [64(b), j=64 (step 0), u=64] broadcast AP."""
    lay = [list(ap64.ap[0]), [0, 64], list(ap64.ap[1])]
    return AP(ap64.tensor, ap64.offset, lay)


def _tree_ap(tmp, n, off_elems):
    """[64 part, (hh 2 x step 2048, j n x step 64, u 64)] at free offset."""
    base = tmp[0:64, :]
    lay = [list(base.ap[0]), [2048, 2], [64, n], [1, 64]]
    return AP(base.tensor, base.offset + off_elems, lay)


def _s2g(ct):  # pe_sem / group count after ct's S1
    return N_PROJ + ct * 160 + 128


def _gend(ct):  # after ct's S2
    return N_PROJ + (ct + 1) * 160


def qkcorr_graph():
    nc = bass.Bass()
    data = nc.dram_tensor("data", [DM, L], BF16, kind="ExternalInput")
    w = nc.dram_tensor("w", [DM, 1536], BF16, kind="ExternalInput")
    wfft = nc.dram_tensor("wfft", [64, 192], BF16, kind="ExternalInput")
    twid = nc.dram_tensor("twid", [128, 128], BF16, kind="ExternalInput")
    out_v = nc.dram_tensor("out_v", [DM, L], BF16, kind="ExternalOutput")
    out_s = nc.dram_tensor("out_s", [2, 8, L], F32, kind="ExternalOutput")
    qdram = nc.dram_tensor("qdram", [DM, L], BF16, kind="Internal")
    kdram = nc.dram_tensor("kdram", [DM, L], BF16, kind="Internal")

    NPS = 8

    from contextlib import ExitStack

    with ExitStack() as stack:
        e = stack.enter_context
        xs = e(nc.sbuf_tensor([128, 4, L], BF16))  # x k-tiles; later Z' planes
        wt = e(nc.sbuf_tensor([128, 4, 1536], BF16))
        evv = e(nc.sbuf_tensor([128, 8, 512], BF16))  # proj staging per mt
        wfs = e(nc.sbuf_tensor([64, 192], BF16))
        tws = e(nc.sbuf_tensor([128, 128], BF16))
        ax = e(nc.sbuf_tensor([128, 128, 64], BF16))  # A_x[a, cc, b] rows 0:64
        gsb = e(nc.sbuf_tensor([128, 2, 64, 2, 64], BF16))  # [b,par,j,pl,u]
        tmp1 = e(nc.sbuf_tensor([128, 4096], F32))
        tmp2 = e(nc.sbuf_tensor([128, 4096], F32))
        xq = e(nc.sbuf_tensor([128, 2, 2, 64, 64], BF16))  # [v,pl,par,j,u]
        xk = e(nc.sbuf_tensor([128, 2, 2, 64, 64], BF16))
        ssum = e(nc.sbuf_tensor([128, 2, 8, 64], F32))  # [v,pl,h,u]
        ps = e(nc.psum_tensor([128, NPS, 512], F32))
        s_w = e(nc.semaphore())
        s_x = e(nc.semaphore())
        s_cw = e(nc.semaphore())
        s_ct = e(nc.semaphore())
        s_stage = e(nc.semaphore())
        s_r1 = e(nc.semaphore())
        pe_sem = e(nc.semaphore())
        dve_sem = e(nc.semaphore())
        tw_sem = e(nc.semaphore())
        cs_sem = e(nc.semaphore())
        s_out = e(nc.semaphore())
        ch_sem = e(nc.semaphore())
        block = e(nc.Block())

        @block.sync
        def _(sync):
            sync.dma_start(
                wt[:, :, :], w.rearrange("(kt p) m -> p kt m", p=128)
            ).then_inc(s_w, 16)
            sync.dma_start(
                xs[:, :, :], data.rearrange("(kt p) n -> p kt n", p=128)
            ).then_inc(s_x, 16)
            sync.dma_start(wfs[:, :], wfft[:, :]).then_inc(s_cw, 16)
            sync.dma_start(tws[:, :], twid[:, :]).then_inc(s_ct, 16)
            # staging: mt 0-3 -> qdram, 4-7 -> kdram, 8-11 -> out_v
            for mt in range(12):
                sync.wait_ge(dve_sem, 8 * (mt + 1))
                dst_t = (qdram, kdram, out_v)[mt // 4]
                r = mt % 4
                dst = dst_t[128 * r : 128 * (r + 1), :].rearrange(
                    "p (ntc c) -> p ntc c", c=512
                )
                sync.dma_start(dst, evv[:, :, :]).then_inc(s_stage, 16)
            # R1 digit-relayout DMAs: ax[a, cc, b] = src[128*chunk+cc, 64a+b]
            for ct, (tname, chunk) in enumerate(ORDER):
                src_t = qdram if tname == "q" else kdram
                sync.wait_ge(s_stage, 16 * (4 if tname == "q" else 8))
                if ct > 0:
                    sync.wait_ge(pe_sem, _s2g(ct - 1))  # ax WAR vs prev S1
                src = src_t[128 * chunk : 128 * (chunk + 1), :].rearrange(
                    "cc (a b) -> a cc b", b=64
                )
                sync.dma_start(ax[0:64, :, :], src).then_inc(s_r1, 16)
            sync.wait_ge(cs_sem, 4)
            sync.dma_start(
                out_s.rearrange("pl h (v u) -> v pl h u", u=64),
                ssum[0:64, :, :, :],
            ).then_inc(s_out, 16)
            # Execution must not complete while output DMAs are in flight:
            # await out_v staging (DMAs 1-12) and the out_s DMA.
            sync.wait_ge(s_stage, 192)
            sync.wait_ge(s_out, 16)

        @block.tensor
        def _(tensor):
            tensor.wait_ge(s_w, 16)
            tensor.wait_ge(s_x, 16)
            g = 0
            for mt in range(12):
                for ntc in range(8):
                    if g >= NPS:
                        tensor.wait_ge(dve_sem, g - NPS + 1)
                    for kt in range(4):
                        mm = nc.tensor.matmul(
                            ps[:, g % NPS, :],
                            wt[:, kt, 128 * mt : 128 * (mt + 1)],
                            xs[:, kt, 512 * ntc : 512 * (ntc + 1)],
                            start=(kt == 0),
                            stop=(kt == 3),
                        )
                        if kt == 3:
                            mm.then_inc(pe_sem, 1)
                    g += 1
            tensor.wait_ge(s_cw, 16)
            for ct, (tname, chunk) in enumerate(ORDER):
                tensor.wait_ge(s_r1, 16 * (ct + 1))
                # S1: psum[b, (pl,u)] = sum_a ax[a, 2j+par, b] * [Wc|Wms][a,:]
                for par in range(2):
                    for j in range(64):
                        if g >= NPS:
                            tensor.wait_ge(dve_sem, g - NPS + 1)
                        nc.tensor.matmul(
                            ps[0:64, g % NPS, 0:128],
                            ax[0:64, 2 * j + par, :],
                            wfs[0:64, 0:128],
                            start=True,
                            stop=True,
                        ).then_inc(pe_sem, 1)
                        g += 1
                tensor.wait_ge(tw_sem, ct + 1)
                # S2: psum[v, 512] ; Z' planes in xs[0:64, 2*pl+par, :]
                for par in range(2):
                    for pch in range(8):
                        sl = slice(512 * pch, 512 * (pch + 1))
                        for pl in range(2):
                            if g >= NPS:
                                tensor.wait_ge(dve_sem, g - NPS + 1)
                            if pl == 0:  # Xr = Wc^T Zr + Ws^T Zi
                                nc.tensor.matmul(
                                    ps[0:64, g % NPS, :],
                                    wfs[0:64, 0:64],
                                    xs[0:64, par, sl],
                                    start=True,
                                    stop=False,
                                )
                                mm = nc.tensor.matmul(
                                    ps[0:64, g % NPS, :],
                                    wfs[0:64, 128:192],
                                    xs[0:64, 2 + par, sl],
                                    start=False,
                                    stop=True,
                                )
                            else:  # Xi = Wc^T Zi + Wms^T Zr
                                nc.tensor.matmul(
                                    ps[0:64, g % NPS, :],
                                    wfs[0:64, 0:64],
                                    xs[0:64, 2 + par, sl],
                                    start=True,
                                    stop=False,
                                )
                                mm = nc.tensor.matmul(
                                    ps[0:64, g % NPS, :],
                                    wfs[0:64, 64:128],
                                    xs[0:64, par, sl],
                                    start=False,
                                    stop=True,
                                )
                            mm.then_inc(pe_sem, 1)
                            g += 1

        @block.vector
        def _(vector):
            g = 0
            nc.vector.memset(ssum[0:64, :, :, :], 0.0).then_inc(ch_sem, 1)
            for mt in range(12):
                if mt >= 1:
                    vector.wait_ge(s_stage, 16 * mt)  # evv WAR vs staging DMA
                for ntc in range(8):
                    vector.wait_ge(pe_sem, g + 1)
                    nc.vector.tensor_copy(evv[:, ntc, :], ps[:, g % NPS, :]).then_inc(
                        dve_sem, 1
                    )
                    g += 1
            vector.wait_ge(s_ct, 16)
            ch = 1  # memset counted

            def chained(mk):
                # Dependent DVE->DVE chains need explicit ordering (engine
                # write visibility); serialize chained ops on ch_sem.
                nonlocal ch
                vector.wait_ge(ch_sem, ch)
                ins = mk()
                ins.then_inc(ch_sem, 1)
                ch += 1
                return ins

            for ct, (tname, chunk) in enumerate(ORDER):
                for par in range(2):
                    for j in range(64):
                        vector.wait_ge(pe_sem, g + 1)
                        nc.vector.tensor_copy(
                            gsb[0:64, par, j, :, :], ps[0:64, g % NPS, 0:128]
                        ).then_inc(dve_sem, 1)
                        g += 1
                # twiddle -> Z' into xs combos; WAR vs prev ct's S2 reads
                if ct > 0:
                    vector.wait_ge(pe_sem, _gend(ct - 1))
                if ct >= 2:
                    # tmp1 WAR vs previous chunk's final CS op
                    vector.wait_ge(cs_sem, ct // 2)
                vector.wait_ge(dve_sem, g)  # gsb evictions visible to DVE
                tc_b = _bcast_j(tws[0:64, 0:64])
                tms_b = _bcast_j(tws[0:64, 64:128])
                t1 = tmp1[0:64, :]
                last = None
                for par in range(2):
                    gr = gsb[0:64, par, :, 0, :]
                    gi = gsb[0:64, par, :, 1, :]
                    zr = xs[0:64, par, :]
                    zi = xs[0:64, 2 + par, :]
                    chained(lambda: nc.vector.scalar_tensor_tensor(zr, gr, 1.0, tc_b, MULT, MULT))
                    chained(lambda: nc.vector.scalar_tensor_tensor(t1, gi, 1.0, tms_b, MULT, MULT))
                    chained(lambda: nc.vector.scalar_tensor_tensor(zr, zr, 1.0, t1, MULT, SUB))
                    chained(lambda: nc.vector.scalar_tensor_tensor(t1, gr, 1.0, tms_b, MULT, MULT))
                    chained(lambda: nc.vector.scalar_tensor_tensor(zi, gi, 1.0, tc_b, MULT, MULT))
                    if par == 0:
                        chained(lambda: nc.vector.scalar_tensor_tensor(zi, zi, 1.0, t1, MULT, ADD))
                    else:
                        # final twiddle op: explicit chain wait, signals tw_sem
                        vector.wait_ge(ch_sem, ch)
                        nc.vector.scalar_tensor_tensor(
                            zi, zi, 1.0, t1, MULT, ADD
                        ).then_inc(tw_sem, 1)
                # S2 evictions
                xdst = xq if tname == "q" else xk
                for par in range(2):
                    for pch in range(8):
                        for pl in range(2):
                            vector.wait_ge(pe_sem, g + 1)
                            nc.vector.tensor_copy(
                                xdst[0:64, pl, par, 8 * pch : 8 * (pch + 1), :],
                                ps[0:64, g % NPS, :],
                            ).then_inc(dve_sem, 1)
                            g += 1
                if tname == "k":
                    # cross-spectrum for this chunk (heads 2c, 2c+1)
                    vector.wait_ge(dve_sem, g)  # X evictions visible to DVE
                    vector.wait_ge(tw_sem, ct + 1)  # final twiddle op done
                    if chunk > 0:
                        vector.wait_ge(cs_sem, chunk)  # ssum WAW vs prev chunk
                    for pl_out, (qa_i, kb_i, qc_i, kd_i, op) in enumerate(
                        [(0, 0, 1, 1, ADD), (1, 0, 0, 1, SUB)]
                    ):
                        for par in range(2):
                            is_final = pl_out == 1 and par == 1
                            qa = xq[0:64, qa_i, par, :, :]
                            kb = xk[0:64, kb_i, par, :, :]
                            qc = xq[0:64, qc_i, par, :, :]
                            kd = xk[0:64, kd_i, par, :, :]
                            t1f = tmp1[0:64, :]
                            t2f = tmp2[0:64, :]
                            chained(lambda: nc.vector.scalar_tensor_tensor(t1f, qa, 1.0, kb, MULT, MULT))
                            chained(lambda: nc.vector.scalar_tensor_tensor(t2f, qc, 1.0, kd, MULT, MULT))
                            chained(lambda: nc.vector.scalar_tensor_tensor(t1f, t1f, 1.0, t2f, MULT, op))
                            n = 16
                            while n >= 1:
                                dst = _tree_ap(tmp1, n, 0)
                                src = _tree_ap(tmp1, n, n * 64)
                                chained(lambda dst=dst, src=src: nc.vector.scalar_tensor_tensor(
                                    dst, dst, 1.0, src, MULT, ADD
                                ))
                                n //= 2
                            for hh in range(2):
                                red = tmp1[0:64, 2048 * hh : 2048 * hh + 64]
                                dst = ssum[0:64, pl_out, 2 * chunk + hh, :]
                                if is_final and hh == 1:
                                    # final CS op: explicit wait, signals cs_sem
                                    vector.wait_ge(ch_sem, ch)
                                    nc.vector.scalar_tensor_tensor(
                                        dst, dst, 1.0, red, MULT, ADD
                                    ).then_inc(cs_sem, 1)
                                else:
                                    chained(lambda red=red, dst=dst: nc.vector.scalar_tensor_tensor(
                                        dst, dst, 1.0, red, MULT, ADD
                                    ))

    return nc


def _get_graphs():
    if not _GRAPHS:
        _GRAPHS["proj"] = qkcorr_graph()
        _GRAPHS["outproj"] = _mm_graph(L, DM)
    return _GRAPHS


LAST_EXEC_NS = [None, None]


def _bf16(a):
    return np.ascontiguousarray(a, dtype=NPBF16)


def kernel(x, Wq, bq, Wk, bk, Wv, bv, Wo, bo):
    x = np.asarray(x, np.float32)
    Wq, bq = np.asarray(Wq, np.float32), np.asarray(bq, np.float32)
    Wk, bk = np.asarray(Wk, np.float32), np.asarray(bk, np.float32)
    Wv, bv = np.asarray(Wv, np.float32), np.asarray(bv, np.float32)
    Wo, bo = np.asarray(Wo, np.float32), np.asarray(bo, np.float32)
    g = _get_graphs()

    # ---- device graph A: QKV projections + on-device cross-spectrum ----
    # (b-sharding on 4 cores dedupes the x upload; Q,K never cross the
    # tunnel - only V and the tiny 8-head spectrum come back.)
    w_all = _bf16(np.concatenate([Wq.T, Wk.T, Wv.T], axis=1))  # (DM, 1536)
    wfft_np, twid_np = fft_consts()
    in_maps = [
        {"data": _bf16(x[b].T), "w": w_all, "wfft": wfft_np, "twid": twid_np}
        for b in range(B)
    ]
    resA = run_bass_kernel_spmd(g["proj"], in_maps, core_ids=list(range(B)))
    LAST_EXEC_NS[0] = resA.exec_time_ns

    # V (channel-major) with bias; spectrum -> corr via host iFFT.
    # Bias on Q,K only shifts corr by a per-head constant (DC bin), which
    # top-k order and softmax are both invariant to, so it is skipped.
    VT = np.empty((B, DM, L), np.float32)
    corr_mean = np.empty((B, H, L), np.float64)
    for b in range(B):
        VT[b] = np.asarray(resA.results[b]["out_v"]).astype(np.float32) + bv[:, None]
        s_ri = np.asarray(resA.results[b]["out_s"])  # (2, 8, L) f32
        S = s_ri[0].astype(np.float64) + 1j * s_ri[1].astype(np.float64)
        corr_mean[b] = np.fft.ifft(S, axis=-1).real / DK

    k = min(int(2 * math.log(L)), L)  # 16
    order = np.argsort(-corr_mean, axis=-1, kind="stable")
    delays = order[..., :k]  # (B, H, k)
    wvals = np.take_along_axis(corr_mean, delays, axis=-1)
    wvals = wvals - wvals.max(axis=-1, keepdims=True)
    wexp = np.exp(wvals)
    wsm = (wexp / wexp.sum(axis=-1, keepdims=True)).astype(np.float32)

    # ctxT[b, c, t] = sum_j w_j * VT[b, c, (t - delay_j) % L]
    ctxT = np.zeros((B, DM, L), np.float32)
    for b in range(B):
        for h in range(H):
            csl = slice(64 * h, 64 * (h + 1))
            acc = ctxT[b, csl]
            for j in range(k):
                d = int(delays[b, h, j])
                wj = wsm[b, h, j]
                rolled = np.concatenate(
                    [VT[b, csl, L - d :], VT[b, csl, : L - d]], axis=1
                )
                acc += wj * rolled
    # ---- device graph B: output projection, one core per batch ----
    woT = _bf16(Wo.T)
    in_maps_b = [{"data": _bf16(ctxT[b]), "w": woT} for b in range(B)]
    resB = run_bass_kernel_spmd(g["outproj"], in_maps_b, core_ids=list(range(B)))
    LAST_EXEC_NS[1] = resB.exec_time_ns

    out = np.empty((B, L, DM), np.float32)
    for b in range(B):
        ob = np.asarray(resB.results[b]["out"]).astype(np.float32)  # (512, L)
        out[b] = ob.T
    out += bo.astype(np.float32)
    return out


# revision 14
# speedup vs baseline: 11.1863x; 1.0124x over previous
"""AutoCorrelation multi-head attention (Autoformer-style) on 8 TRN2 NeuronCores.

Shapes (hardcoded): B=4, L=4096, DM=512, H=8, Dk=64, k=16.

The axon tunnel to the devices moves ~30-60MB/s with ~0.2-0.5s fixed cost
per dispatch, so the design minimizes host<->device bytes and call count:
two device calls, all transfers bf16 (tolerance 2e-2; we land ~1.2e-2),
and Q/K never cross the tunnel at all - the FFT cross-correlation spectrum
is computed ON DEVICE (Cooley-Tukey 4096 = 64x64, both 64-point DFT stages
as PE matmuls against host-uploaded constant matrices, twiddle and
cross-spectrum reduction on the vector engine). Only V (4MB bf16) and the
cross-spectrum S (256KB f32) come back per core.

Sharding: one core per batch element (4 of 8 cores; the tunnel, not
device compute, is the bottleneck, and b-sharding dedupes the x upload).

Device graph A (per core): QKV projections (bf16 matmuls, f32 psum) +
S_h[f] = sum_d Qf conj(Kf) per head. Bias on Q/K only shifts the corr by
a per-head constant (DC bin), which top-k order and softmax are invariant
to, so the spectrum uses pre-bias Q,K.
Host: tiny iFFT of S (8 x 4096 per b), top-k(16), softmax, rolled gather
of V (16 shifted accumulations), bias adds.
Device graph B (per core): output projection ctx^T @ Wo^T for its batch.

Raw bass with manual semaphores (this walrus build allows only ONE
embedded sync wait per instruction, so Tile's multi-wait tail drain cannot
compile; multi-waits are standalone wait_ge's). Race-free by construction:
one DMA per dram tensor on its own semaphore, in-order engine queues
counted exactly, dependent same-engine DVE chains serialized on a chain
semaphore, and each graph's sync program ends by waiting for its output
DMAs to complete (without this, execution can be declared done while the
last DMA is in flight - observed as warm-run corruption).
"""

import os
import sys
import math

for _p in ("/opt/trn_rl_repo",):
    if os.path.isdir(_p) and _p not in sys.path:
        sys.path.insert(0, _p)

import numpy as np
import ml_dtypes

import concourse.bass as bass
import concourse.mybir as mybir
from concourse.bass import AP
from concourse.bass_utils import run_bass_kernel_spmd

B, L, DM, H, DK = 4, 4096, 512, 8, 64
KTOP = 16
N_CORES = 8
F32 = mybir.dt.float32
BF16 = mybir.dt.bfloat16
NPBF16 = ml_dtypes.bfloat16

_GRAPHS = {}


def _mm_graph(n_dim, w_cols):
    """out[w_cols, n_dim] = w.T @ data  (bf16 in, f32 psum accum, bf16 out).

    data [DM=512, n_dim], w [DM, w_cols].

    Raw bass (this walrus build allows only ONE sync wait per instruction,
    so Tile's multi-wait tail drain cannot compile; multi-waits here are
    standalone wait_ge instructions). Race-free by construction: exactly one
    DMA per dram tensor, each completing on its own semaphore, so every
    wait threshold identifies a unique DMA's completion (the old graph
    counted completions of many DMAs on one semaphore, which assumed
    cross-queue in-order completion - the source of the rare corruption).
    pe_sem/dve_sem count single in-order engine queues, which is exact.
    """
    nc = bass.Bass()
    data = nc.dram_tensor("data", [DM, n_dim], BF16, kind="ExternalInput")
    w = nc.dram_tensor("w", [DM, w_cols], BF16, kind="ExternalInput")
    out = nc.dram_tensor("out", [w_cols, n_dim], BF16, kind="ExternalOutput")

    kt_n = DM // 128  # 4 contraction tiles
    mt_n = w_cols // 128  # output-row tiles
    nc_n = n_dim // 512  # output-col chunks
    n_groups = mt_n * nc_n
    NPS = 8  # psum banks cycled

    with (
        nc.sbuf_tensor([128, kt_n, n_dim], BF16) as xs,
        nc.sbuf_tensor([128, kt_n, w_cols], BF16) as wt,
        nc.sbuf_tensor([128, mt_n, nc_n, 512], BF16) as ev,
        nc.psum_tensor([128, NPS, 512], F32) as ps,
        nc.semaphore() as w_sem,
        nc.semaphore() as x_sem,
        nc.semaphore() as pe_sem,
        nc.semaphore() as dve_sem,
        nc.semaphore() as odma_sem,
        nc.Block() as block,
    ):

        @block.sync
        def _(sync):
            sync.dma_start(wt[:, :, :], w.rearrange("(kt p) m -> p kt m", p=128)).then_inc(
                w_sem, 16
            )
            sync.dma_start(xs[:, :, :], data.rearrange("(kt p) n -> p kt n", p=128)).then_inc(
                x_sem, 16
            )
            sync.wait_ge(dve_sem, n_groups)
            sync.dma_start(
                out.rearrange("(mt p) (ntc c) -> p mt ntc c", p=128, c=512),
                ev[:, :, :, :],
            ).then_inc(odma_sem, 16)
            sync.wait_ge(odma_sem, 16)

        @block.tensor
        def _(tensor):
            tensor.wait_ge(w_sem, 16)
            tensor.wait_ge(x_sem, 16)
            for g in range(n_groups):
                mt, ntc = divmod(g, nc_n)
                if g >= NPS:
                    tensor.wait_ge(dve_sem, g - NPS + 1)
                for kt in range(kt_n):
                    mm = nc.tensor.matmul(
                        ps[:, g % NPS, :],
                        wt[:, kt, 128 * mt : 128 * (mt + 1)],
                        xs[:, kt, 512 * ntc : 512 * (ntc + 1)],
                        start=(kt == 0),
                        stop=(kt == kt_n - 1),
                    )
                    if kt == kt_n - 1:
                        mm.then_inc(pe_sem, 1)

        @block.vector
        def _(vector):
            for g in range(n_groups):
                mt, ntc = divmod(g, nc_n)
                vector.wait_ge(pe_sem, g + 1)
                nc.vector.tensor_copy(ev[:, mt, ntc, :], ps[:, g % NPS, :]).then_inc(
                    dve_sem, 1
                )

    return nc



MULT = mybir.AluOpType.mult
ADD = mybir.AluOpType.add
SUB = mybir.AluOpType.subtract

ORDER = [("q", 0), ("k", 0), ("q", 1), ("k", 1), ("q", 2), ("k", 2), ("q", 3), ("k", 3)]
N_PROJ = 96  # proj psum groups; then 160 per (tensor, chunk): 128 S1 + 32 S2


def fft_consts():
    j = np.arange(64)
    wc = np.cos(2 * np.pi * np.outer(j, j) / 64)
    ws = np.sin(2 * np.pi * np.outer(j, j) / 64)
    wfft = np.concatenate([wc, -ws, ws], axis=1)  # [Wc | Wms | Ws] (64, 192)
    tc = np.cos(2 * np.pi * np.outer(j, j) / L)  # [b, u]
    tms = -np.sin(2 * np.pi * np.outer(j, j) / L)
    twid = np.zeros((128, 128), np.float32)
    twid[0:64, 0:64] = tc
    twid[0:64, 64:128] = tms
    return (
        np.ascontiguousarray(wfft, NPBF16),
        np.ascontiguousarray(twid, NPBF16),
    )


def _bcast_j(ap64):
    """[64(b), 64(u)] -> [64(b), j=64 (step 0), u=64] broadcast AP."""
    lay = [list(ap64.ap[0]), [0, 64], list(ap64.ap[1])]
    return AP(ap64.tensor, ap64.offset, lay)


def _tree_ap(tmp, n, off_elems):
    """[64 part, (hh 2 x step 2048, j n x step 64, u 64)] at free offset."""
    base = tmp[0:64, :]
    lay = [list(base.ap[0]), [2048, 2], [64, n], [1, 64]]
    return AP(base.tensor, base.offset + off_elems, lay)


def _s2g(ct):  # pe_sem / group count after ct's S1
    return N_PROJ + ct * 160 + 128


def _gend(ct):  # after ct's S2
    return N_PROJ + (ct + 1) * 160


def qkcorr_graph():
    nc = bass.Bass()
    data = nc.dram_tensor("data", [DM, L], BF16, kind="ExternalInput")
    w = nc.dram_tensor("w", [DM, 1536], BF16, kind="ExternalInput")
    wfft = nc.dram_tensor("wfft", [64, 192], BF16, kind="ExternalInput")
    twid = nc.dram_tensor("twid", [128, 128], BF16, kind="ExternalInput")
    out_v = nc.dram_tensor("out_v", [DM, L], BF16, kind="ExternalOutput")
    out_s = nc.dram_tensor("out_s", [2, 8, L], F32, kind="ExternalOutput")
    qdram = nc.dram_tensor("qdram", [DM, L], BF16, kind="Internal")
    kdram = nc.dram_tensor("kdram", [DM, L], BF16, kind="Internal")

    NPS = 8

    from contextlib import ExitStack

    with ExitStack() as stack:
        e = stack.enter_context
        xs = e(nc.sbuf_tensor([128, 4, L], BF16))  # x k-tiles; later Z' planes
        wt = e(nc.sbuf_tensor([128, 4, 1536], BF16))
        evv = e(nc.sbuf_tensor([128, 8, 512], BF16))  # proj staging per mt
        wfs = e(nc.sbuf_tensor([64, 192], BF16))
        tws = e(nc.sbuf_tensor([128, 128], BF16))
        ax = e(nc.sbuf_tensor([128, 128, 64], BF16))  # A_x[a, cc, b] rows 0:64
        gsb = e(nc.sbuf_tensor([128, 2, 64, 2, 64], BF16))  # [b,par,j,pl,u]
        tmp1 = e(nc.sbuf_tensor([128, 4096], F32))
        tmp2 = e(nc.sbuf_tensor([128, 4096], F32))
        xq = e(nc.sbuf_tensor([128, 2, 2, 64, 64], BF16))  # [v,pl,par,j,u]
        xk = e(nc.sbuf_tensor([128, 2, 2, 64, 64], BF16))
        ssum = e(nc.sbuf_tensor([128, 2, 8, 64], F32))  # [v,pl,h,u]
        ps = e(nc.psum_tensor([128, NPS, 512], F32))
        s_w = e(nc.semaphore())
        s_x = e(nc.semaphore())
        s_cw = e(nc.semaphore())
        s_ct = e(nc.semaphore())
        s_stage = e(nc.semaphore())
        s_r1 = e(nc.semaphore())
        pe_sem = e(nc.semaphore())
        dve_sem = e(nc.semaphore())
        tw_sem = e(nc.semaphore())
        cs_sem = e(nc.semaphore())
        s_out = e(nc.semaphore())
        ch_sem = e(nc.semaphore())
        block = e(nc.Block())

        @block.sync
        def _(sync):
            sync.dma_start(
                wt[:, :, :], w.rearrange("(kt p) m -> p kt m", p=128)
            ).then_inc(s_w, 16)
            sync.dma_start(
                xs[:, :, :], data.rearrange("(kt p) n -> p kt n", p=128)
            ).then_inc(s_x, 16)
            sync.dma_start(wfs[:, :], wfft[:, :]).then_inc(s_cw, 16)
            sync.dma_start(tws[:, :], twid[:, :]).then_inc(s_ct, 16)
            # staging: mt 0-3 -> qdram, 4-7 -> kdram, 8-11 -> out_v
            for mt in range(12):
                sync.wait_ge(dve_sem, 8 * (mt + 1))
                dst_t = (qdram, kdram, out_v)[mt // 4]
                r = mt % 4
                dst = dst_t[128 * r : 128 * (r + 1), :].rearrange(
                    "p (ntc c) -> p ntc c", c=512
                )
                sync.dma_start(dst, evv[:, :, :]).then_inc(s_stage, 16)
            # R1 digit-relayout DMAs: ax[a, cc, b] = src[128*chunk+cc, 64a+b]
            for ct, (tname, chunk) in enumerate(ORDER):
                src_t = qdram if tname == "q" else kdram
                sync.wait_ge(s_stage, 16 * (4 if tname == "q" else 8))
                if ct > 0:
                    sync.wait_ge(pe_sem, _s2g(ct - 1))  # ax WAR vs prev S1
                src = src_t[128 * chunk : 128 * (chunk + 1), :].rearrange(
                    "cc (a b) -> a cc b", b=64
                )
                sync.dma_start(ax[0:64, :, :], src).then_inc(s_r1, 16)
            sync.wait_ge(cs_sem, 4)
            sync.dma_start(
                out_s.rearrange("pl h (v u) -> v pl h u", u=64),
                ssum[0:64, :, :, :],
            ).then_inc(s_out, 16)
            # Execution must not complete while output DMAs are in flight:
            # await out_v staging (DMAs 1-12) and the out_s DMA.
            sync.wait_ge(s_stage, 192)
            sync.wait_ge(s_out, 16)

        @block.tensor
        def _(tensor):
            tensor.wait_ge(s_w, 16)
            tensor.wait_ge(s_x, 16)
            g = 0
            for mt in range(12):
                for ntc in range(8):
                    if g >= NPS:
                        tensor.wait_ge(dve_sem, g - NPS + 1)
                    for kt in range(4):
                        mm = nc.tensor.matmul(
                            ps[:, g % NPS, :],
                            wt[:, kt, 128 * mt : 128 * (mt + 1)],
                            xs[:, kt, 512 * ntc : 512 * (ntc + 1)],
                            start=(kt == 0),
                            stop=(kt == 3),
                        )
                        if kt == 3:
                            mm.then_inc(pe_sem, 1)
                    g += 1
            tensor.wait_ge(s_cw, 16)
            for ct, (tname, chunk) in enumerate(ORDER):
                tensor.wait_ge(s_r1, 16 * (ct + 1))
                # S1: psum[b, (pl,u)] = sum_a ax[a, 2j+par, b] * [Wc|Wms][a,:]
                for par in range(2):
                    for j in range(64):
                        if g >= NPS:
                            tensor.wait_ge(dve_sem, g - NPS + 1)
                        nc.tensor.matmul(
                            ps[0:64, g % NPS, 0:128],
                            ax[0:64, 2 * j + par, :],
                            wfs[0:64, 0:128],
                            start=True,
                            stop=True,
                        ).then_inc(pe_sem, 1)
                        g += 1
                tensor.wait_ge(tw_sem, ct + 1)
                # S2: psum[v, 512] ; Z' planes in xs[0:64, 2*pl+par, :]
                for par in range(2):
                    for pch in range(8):
                        sl = slice(512 * pch, 512 * (pch + 1))
                        for pl in range(2):
                            if g >= NPS:
                                tensor.wait_ge(dve_sem, g - NPS + 1)
                            if pl == 0:  # Xr = Wc^T Zr + Ws^T Zi
                                nc.tensor.matmul(
                                    ps[0:64, g % NPS, :],
                                    wfs[0:64, 0:64],
                                    xs[0:64, par, sl],
                                    start=True,
                                    stop=False,
                                )
                                mm = nc.tensor.matmul(
                                    ps[0:64, g % NPS, :],
                                    wfs[0:64, 128:192],
                                    xs[0:64, 2 + par, sl],
                                    start=False,
                                    stop=True,
                                )
                            else:  # Xi = Wc^T Zi + Wms^T Zr
                                nc.tensor.matmul(
                                    ps[0:64, g % NPS, :],
                                    wfs[0:64, 0:64],
                                    xs[0:64, 2 + par, sl],
                                    start=True,
                                    stop=False,
                                )
                                mm = nc.tensor.matmul(
                                    ps[0:64, g % NPS, :],
                                    wfs[0:64, 64:128],
                                    xs[0:64, par, sl],
                                    start=False,
                                    stop=True,
                                )
                            mm.then_inc(pe_sem, 1)
                            g += 1

        @block.vector
        def _(vector):
            g = 0
            nc.vector.memset(ssum[0:64, :, :, :], 0.0).then_inc(ch_sem, 1)
            for mt in range(12):
                if mt >= 1:
                    vector.wait_ge(s_stage, 16 * mt)  # evv WAR vs staging DMA
                for ntc in range(8):
                    vector.wait_ge(pe_sem, g + 1)
                    nc.vector.tensor_copy(evv[:, ntc, :], ps[:, g % NPS, :]).then_inc(
                        dve_sem, 1
                    )
                    g += 1
            vector.wait_ge(s_ct, 16)
            ch = 1  # memset counted

            def chained(mk):
                # Dependent DVE->DVE chains need explicit ordering (engine
                # write visibility); serialize chained ops on ch_sem.
                nonlocal ch
                vector.wait_ge(ch_sem, ch)
                ins = mk()
                ins.then_inc(ch_sem, 1)
                ch += 1
                return ins

            for ct, (tname, chunk) in enumerate(ORDER):
                for par in range(2):
                    for j in range(64):
                        vector.wait_ge(pe_sem, g + 1)
                        nc.vector.tensor_copy(
                            gsb[0:64, par, j, :, :], ps[0:64, g % NPS, 0:128]
                        ).then_inc(dve_sem, 1)
                        g += 1
                # twiddle -> Z' into xs combos; WAR vs prev ct's S2 reads
                if ct > 0:
                    vector.wait_ge(pe_sem, _gend(ct - 1))
                if ct >= 2:
                    # tmp1 WAR vs previous chunk's final CS op
                    vector.wait_ge(cs_sem, ct // 2)
                vector.wait_ge(dve_sem, g)  # gsb evictions visible to DVE
                tc_b = _bcast_j(tws[0:64, 0:64])
                tms_b = _bcast_j(tws[0:64, 64:128])
                t1 = tmp1[0:64, :]
                last = None
                for par in range(2):
                    gr = gsb[0:64, par, :, 0, :]
                    gi = gsb[0:64, par, :, 1, :]
                    zr = xs[0:64, par, :]
                    zi = xs[0:64, 2 + par, :]
                    chained(lambda: nc.vector.scalar_tensor_tensor(zr, gr, 1.0, tc_b, MULT, MULT))
                    chained(lambda: nc.vector.scalar_tensor_tensor(t1, gi, 1.0, tms_b, MULT, MULT))
                    chained(lambda: nc.vector.scalar_tensor_tensor(zr, zr, 1.0, t1, MULT, SUB))
                    chained(lambda: nc.vector.scalar_tensor_tensor(t1, gr, 1.0, tms_b, MULT, MULT))
                    chained(lambda: nc.vector.scalar_tensor_tensor(zi, gi, 1.0, tc_b, MULT, MULT))
                    if par == 0:
                        chained(lambda: nc.vector.scalar_tensor_tensor(zi, zi, 1.0, t1, MULT, ADD))
                    else:
                        # final twiddle op: explicit chain wait, signals tw_sem
                        vector.wait_ge(ch_sem, ch)
                        nc.vector.scalar_tensor_tensor(
                            zi, zi, 1.0, t1, MULT, ADD
                        ).then_inc(tw_sem, 1)
                # S2 evictions
                xdst = xq if tname == "q" else xk
                for par in range(2):
                    for pch in range(8):
                        for pl in range(2):
                            vector.wait_ge(pe_sem, g + 1)
                            nc.vector.tensor_copy(
                                xdst[0:64, pl, par, 8 * pch : 8 * (pch + 1), :],
                                ps[0:64, g % NPS, :],
                            ).then_inc(dve_sem, 1)
                            g += 1
                if tname == "k":
                    # cross-spectrum for this chunk (heads 2c, 2c+1)
                    vector.wait_ge(dve_sem, g)  # X evictions visible to DVE
                    vector.wait_ge(tw_sem, ct + 1)  # final twiddle op done
                    if chunk > 0:
                        vector.wait_ge(cs_sem, chunk)  # ssum WAW vs prev chunk
                    for pl_out, (qa_i, kb_i, qc_i, kd_i, op) in enumerate(
                        [(0, 0, 1, 1, ADD), (1, 0, 0, 1, SUB)]
                    ):
                        for par in range(2):
                            is_final = pl_out == 1 and par == 1
                            qa = xq[0:64, qa_i, par, :, :]
                            kb = xk[0:64, kb_i, par, :, :]
                            qc = xq[0:64, qc_i, par, :, :]
                            kd = xk[0:64, kd_i, par, :, :]
                            t1f = tmp1[0:64, :]
                            t2f = tmp2[0:64, :]
                            chained(lambda: nc.vector.scalar_tensor_tensor(t1f, qa, 1.0, kb, MULT, MULT))
                            chained(lambda: nc.vector.scalar_tensor_tensor(t2f, qc, 1.0, kd, MULT, MULT))
                            chained(lambda: nc.vector.scalar_tensor_tensor(t1f, t1f, 1.0, t2f, MULT, op))
                            n = 16
                            while n >= 1:
                                dst = _tree_ap(tmp1, n, 0)
                                src = _tree_ap(tmp1, n, n * 64)
                                chained(lambda dst=dst, src=src: nc.vector.scalar_tensor_tensor(
                                    dst, dst, 1.0, src, MULT, ADD
                                ))
                                n //= 2
                            for hh in range(2):
                                red = tmp1[0:64, 2048 * hh : 2048 * hh + 64]
                                dst = ssum[0:64, pl_out, 2 * chunk + hh, :]
                                if is_final and hh == 1:
                                    # final CS op: explicit wait, signals cs_sem
                                    vector.wait_ge(ch_sem, ch)
                                    nc.vector.scalar_tensor_tensor(
                                        dst, dst, 1.0, red, MULT, ADD
                                    ).then_inc(cs_sem, 1)
                                else:
                                    chained(lambda red=red, dst=dst: nc.vector.scalar_tensor_tensor(
                                        dst, dst, 1.0, red, MULT, ADD
                                    ))

    return nc


def _get_graphs():
    if not _GRAPHS:
        _GRAPHS["proj"] = qkcorr_graph()
        _GRAPHS["outproj"] = _mm_graph(L, DM)
    return _GRAPHS


LAST_EXEC_NS = [None, None]


def _bf16(a):
    return np.ascontiguousarray(a, dtype=NPBF16)


def kernel(x, Wq, bq, Wk, bk, Wv, bv, Wo, bo):
    x = np.asarray(x, np.float32)
    Wq, bq = np.asarray(Wq, np.float32), np.asarray(bq, np.float32)
    Wk, bk = np.asarray(Wk, np.float32), np.asarray(bk, np.float32)
    Wv, bv = np.asarray(Wv, np.float32), np.asarray(bv, np.float32)
    Wo, bo = np.asarray(Wo, np.float32), np.asarray(bo, np.float32)
    g = _get_graphs()

    # ---- device graph A: QKV projections + on-device cross-spectrum ----
    # (b-sharding on 4 cores dedupes the x upload; Q,K never cross the
    # tunnel - only V and the tiny 8-head spectrum come back.)
    w_all = _bf16(np.concatenate([Wq.T, Wk.T, Wv.T], axis=1))  # (DM, 1536)
    wfft_np, twid_np = fft_consts()
    in_maps = [
        {"data": _bf16(x[b].T), "w": w_all, "wfft": wfft_np, "twid": twid_np}
        for b in range(B)
    ]
    resA = run_bass_kernel_spmd(g["proj"], in_maps, core_ids=list(range(B)))
    LAST_EXEC_NS[0] = resA.exec_time_ns

    # V (channel-major) with bias; spectrum -> corr via host iFFT.
    # Bias on Q,K only shifts corr by a per-head constant (DC bin), which
    # top-k order and softmax are both invariant to, so it is skipped.
    VT = np.empty((B, DM, L), np.float32)
    corr_mean = np.empty((B, H, L), np.float64)
    for b in range(B):
        VT[b] = np.asarray(resA.results[b]["out_v"]).astype(np.float32) + bv[:, None]
        s_ri = np.asarray(resA.results[b]["out_s"])  # (2, 8, L) f32
        S = s_ri[0].astype(np.float64) + 1j * s_ri[1].astype(np.float64)
        corr_mean[b] = np.fft.ifft(S, axis=-1).real / DK

    k = min(int(2 * math.log(L)), L)  # 16
    order = np.argsort(-corr_mean, axis=-1, kind="stable")
    delays = order[..., :k]  # (B, H, k)
    wvals = np.take_along_axis(corr_mean, delays, axis=-1)
    wvals = wvals - wvals.max(axis=-1, keepdims=True)
    wexp = np.exp(wvals)
    wsm = (wexp / wexp.sum(axis=-1, keepdims=True)).astype(np.float32)

    # ctxT[b, c, t] = sum_j w_j * VT[b, c, (t - delay_j) % L]
    ctxT = np.zeros((B, DM, L), np.float32)
    for b in range(B):
        for h in range(H):
            csl = slice(64 * h, 64 * (h + 1))
            acc = ctxT[b, csl]
            for j in range(k):
                d = int(delays[b, h, j])
                wj = wsm[b, h, j]
                rolled = np.concatenate(
                    [VT[b, csl, L - d :], VT[b, csl, : L - d]], axis=1
                )
                acc += wj * rolled
    # ---- device graph B: output projection, one core per batch ----
    woT = _bf16(Wo.T)
    in_maps_b = [{"data": _bf16(ctxT[b]), "w": woT} for b in range(B)]
    resB = run_bass_kernel_spmd(g["outproj"], in_maps_b, core_ids=list(range(B)))
    LAST_EXEC_NS[1] = resB.exec_time_ns

    out = np.empty((B, L, DM), np.float32)
    for b in range(B):
        ob = np.asarray(resB.results[b]["out"]).astype(np.float32)  # (512, L)
        out[b] = ob.T
    out += bo.astype(np.float32)
    return out


# revision 15
# speedup vs baseline: 15.0995x; 1.3498x over previous
"""AutoCorrelation multi-head attention (Autoformer-style) on 8 TRN2 NeuronCores.

Shapes (hardcoded): B=4, L=4096, DM=512, H=8, Dk=64, k=16.

The axon tunnel to the devices moves ~30-60MB/s with ~0.2-0.5s fixed cost
per dispatch, so the design minimizes host<->device bytes and call count:
two device calls, all transfers bf16 (tolerance 2e-2; we land ~1.2e-2),
and Q/K never cross the tunnel at all - the FFT cross-correlation spectrum
is computed ON DEVICE (Cooley-Tukey 4096 = 64x64, both 64-point DFT stages
as PE matmuls against host-uploaded constant matrices, twiddle and
cross-spectrum reduction on the vector engine). Only V (4MB bf16) and the
cross-spectrum S (256KB f32) come back per core.

Sharding: one core per batch element (4 of 8 cores; the tunnel, not
device compute, is the bottleneck, and b-sharding dedupes the x upload).

Device graph A (per core): QKV projections (bf16 matmuls, f32 psum) +
S_h[f] = sum_d Qf conj(Kf) per head. Bias on Q/K only shifts the corr by
a per-head constant (DC bin), which top-k order and softmax are invariant
to, so the spectrum uses pre-bias Q,K.
Host: tiny iFFT of S (8 x 4096 per b), top-k(16), softmax, rolled gather
of V (16 shifted accumulations), bias adds.
Device graph B (per core): output projection ctx^T @ Wo^T for its batch.

Raw bass with manual semaphores (this walrus build allows only ONE
embedded sync wait per instruction, so Tile's multi-wait tail drain cannot
compile; multi-waits are standalone wait_ge's). Race-free by construction:
one DMA per dram tensor on its own semaphore, in-order engine queues
counted exactly, dependent same-engine DVE chains serialized on a chain
semaphore, and each graph's sync program ends by waiting for its output
DMAs to complete (without this, execution can be declared done while the
last DMA is in flight - observed as warm-run corruption).
"""

import os
import sys
import math

for _p in ("/opt/trn_rl_repo",):
    if os.path.isdir(_p) and _p not in sys.path:
        sys.path.insert(0, _p)

import jax

try:
    # run_bass_via_pjrt builds a fresh jit closure per call, so every device
    # dispatch pays a full XLA recompile (~0.15s) without this; the
    # persistent cache turns it into a disk hit.
    jax.config.update("jax_compilation_cache_dir", "/tmp/jaxcache")
    jax.config.update("jax_persistent_cache_min_entry_size_bytes", -1)
    jax.config.update("jax_persistent_cache_min_compile_time_secs", 0)
except Exception:
    pass

import numpy as np
import ml_dtypes

import concourse.bass as bass
import concourse.mybir as mybir
from concourse.bass import AP
from concourse.bass_utils import run_bass_kernel_spmd

B, L, DM, H, DK = 4, 4096, 512, 8, 64
KTOP = 16
N_CORES = 8
F32 = mybir.dt.float32
BF16 = mybir.dt.bfloat16
NPBF16 = ml_dtypes.bfloat16

_GRAPHS = {}


def _mm_graph(n_dim, w_cols):
    """out[w_cols, n_dim] = w.T @ data  (bf16 in, f32 psum accum, bf16 out).

    data [DM=512, n_dim], w [DM, w_cols].

    Raw bass (this walrus build allows only ONE sync wait per instruction,
    so Tile's multi-wait tail drain cannot compile; multi-waits here are
    standalone wait_ge instructions). Race-free by construction: exactly one
    DMA per dram tensor, each completing on its own semaphore, so every
    wait threshold identifies a unique DMA's completion (the old graph
    counted completions of many DMAs on one semaphore, which assumed
    cross-queue in-order completion - the source of the rare corruption).
    pe_sem/dve_sem count single in-order engine queues, which is exact.
    """
    nc = bass.Bass()
    data = nc.dram_tensor("data", [DM, n_dim], BF16, kind="ExternalInput")
    w = nc.dram_tensor("w", [DM, w_cols], BF16, kind="ExternalInput")
    out = nc.dram_tensor("out", [w_cols, n_dim], BF16, kind="ExternalOutput")

    kt_n = DM // 128  # 4 contraction tiles
    mt_n = w_cols // 128  # output-row tiles
    nc_n = n_dim // 512  # output-col chunks
    n_groups = mt_n * nc_n
    NPS = 8  # psum banks cycled

    with (
        nc.sbuf_tensor([128, kt_n, n_dim], BF16) as xs,
        nc.sbuf_tensor([128, kt_n, w_cols], BF16) as wt,
        nc.sbuf_tensor([128, mt_n, nc_n, 512], BF16) as ev,
        nc.psum_tensor([128, NPS, 512], F32) as ps,
        nc.semaphore() as w_sem,
        nc.semaphore() as x_sem,
        nc.semaphore() as pe_sem,
        nc.semaphore() as dve_sem,
        nc.semaphore() as odma_sem,
        nc.Block() as block,
    ):

        @block.sync
        def _(sync):
            sync.dma_start(wt[:, :, :], w.rearrange("(kt p) m -> p kt m", p=128)).then_inc(
                w_sem, 16
            )
            sync.dma_start(xs[:, :, :], data.rearrange("(kt p) n -> p kt n", p=128)).then_inc(
                x_sem, 16
            )
            sync.wait_ge(dve_sem, n_groups)
            sync.dma_start(
                out.rearrange("(mt p) (ntc c) -> p mt ntc c", p=128, c=512),
                ev[:, :, :, :],
            ).then_inc(odma_sem, 16)
            sync.wait_ge(odma_sem, 16)

        @block.tensor
        def _(tensor):
            tensor.wait_ge(w_sem, 16)
            tensor.wait_ge(x_sem, 16)
            for g in range(n_groups):
                mt, ntc = divmod(g, nc_n)
                if g >= NPS:
                    tensor.wait_ge(dve_sem, g - NPS + 1)
                for kt in range(kt_n):
                    mm = nc.tensor.matmul(
                        ps[:, g % NPS, :],
                        wt[:, kt, 128 * mt : 128 * (mt + 1)],
                        xs[:, kt, 512 * ntc : 512 * (ntc + 1)],
                        start=(kt == 0),
                        stop=(kt == kt_n - 1),
                    )
                    if kt == kt_n - 1:
                        mm.then_inc(pe_sem, 1)

        @block.vector
        def _(vector):
            for g in range(n_groups):
                mt, ntc = divmod(g, nc_n)
                vector.wait_ge(pe_sem, g + 1)
                nc.vector.tensor_copy(ev[:, mt, ntc, :], ps[:, g % NPS, :]).then_inc(
                    dve_sem, 1
                )

    return nc



MULT = mybir.AluOpType.mult
ADD = mybir.AluOpType.add
SUB = mybir.AluOpType.subtract

ORDER = [("q", 0), ("k", 0), ("q", 1), ("k", 1), ("q", 2), ("k", 2), ("q", 3), ("k", 3)]
N_PROJ = 96  # proj psum groups; then 160 per (tensor, chunk): 128 S1 + 32 S2


def fft_consts():
    j = np.arange(64)
    wc = np.cos(2 * np.pi * np.outer(j, j) / 64)
    ws = np.sin(2 * np.pi * np.outer(j, j) / 64)
    wfft = np.concatenate([wc, -ws, ws], axis=1)  # [Wc | Wms | Ws] (64, 192)
    tc = np.cos(2 * np.pi * np.outer(j, j) / L)  # [b, u]
    tms = -np.sin(2 * np.pi * np.outer(j, j) / L)
    twid = np.zeros((128, 128), np.float32)
    twid[0:64, 0:64] = tc
    twid[0:64, 64:128] = tms
    return (
        np.ascontiguousarray(wfft, NPBF16),
        np.ascontiguousarray(twid, NPBF16),
    )


def _bcast_j(ap64):
    """[64(b), 64(u)] -> [64(b), j=64 (step 0), u=64] broadcast AP."""
    lay = [list(ap64.ap[0]), [0, 64], list(ap64.ap[1])]
    return AP(ap64.tensor, ap64.offset, lay)


def _tree_ap(tmp, n, off_elems):
    """[64 part, (hh 2 x step 2048, j n x step 64, u 64)] at free offset."""
    base = tmp[0:64, :]
    lay = [list(base.ap[0]), [2048, 2], [64, n], [1, 64]]
    return AP(base.tensor, base.offset + off_elems, lay)


def _s2g(ct):  # pe_sem / group count after ct's S1
    return N_PROJ + ct * 160 + 128


def _gend(ct):  # after ct's S2
    return N_PROJ + (ct + 1) * 160


def qkcorr_graph():
    nc = bass.Bass()
    data = nc.dram_tensor("data", [DM, L], BF16, kind="ExternalInput")
    w = nc.dram_tensor("w", [DM, 1536], BF16, kind="ExternalInput")
    wfft = nc.dram_tensor("wfft", [64, 192], BF16, kind="ExternalInput")
    twid = nc.dram_tensor("twid", [128, 128], BF16, kind="ExternalInput")
    out_v = nc.dram_tensor("out_v", [DM, L], BF16, kind="ExternalOutput")
    out_s = nc.dram_tensor("out_s", [2, 8, L], F32, kind="ExternalOutput")
    qdram = nc.dram_tensor("qdram", [DM, L], BF16, kind="Internal")
    kdram = nc.dram_tensor("kdram", [DM, L], BF16, kind="Internal")

    NPS = 8

    from contextlib import ExitStack

    with ExitStack() as stack:
        e = stack.enter_context
        xs = e(nc.sbuf_tensor([128, 4, L], BF16))  # x k-tiles; later Z' planes
        wt = e(nc.sbuf_tensor([128, 4, 1536], BF16))
        evv = e(nc.sbuf_tensor([128, 8, 512], BF16))  # proj staging per mt
        wfs = e(nc.sbuf_tensor([64, 192], BF16))
        tws = e(nc.sbuf_tensor([128, 128], BF16))
        ax = e(nc.sbuf_tensor([128, 128, 64], BF16))  # A_x[a, cc, b] rows 0:64
        gsb = e(nc.sbuf_tensor([128, 2, 64, 2, 64], BF16))  # [b,par,j,pl,u]
        tmp1 = e(nc.sbuf_tensor([128, 4096], F32))
        tmp2 = e(nc.sbuf_tensor([128, 4096], F32))
        xq = e(nc.sbuf_tensor([128, 2, 2, 64, 64], BF16))  # [v,pl,par,j,u]
        xk = e(nc.sbuf_tensor([128, 2, 2, 64, 64], BF16))
        ssum = e(nc.sbuf_tensor([128, 2, 8, 64], F32))  # [v,pl,h,u]
        ps = e(nc.psum_tensor([128, NPS, 512], F32))
        s_w = e(nc.semaphore())
        s_x = e(nc.semaphore())
        s_cw = e(nc.semaphore())
        s_ct = e(nc.semaphore())
        s_stage = e(nc.semaphore())
        s_r1 = e(nc.semaphore())
        pe_sem = e(nc.semaphore())
        dve_sem = e(nc.semaphore())
        tw_sem = e(nc.semaphore())
        cs_sem = e(nc.semaphore())
        s_out = e(nc.semaphore())
        ch_sem = e(nc.semaphore())
        block = e(nc.Block())

        @block.sync
        def _(sync):
            sync.dma_start(
                wt[:, :, :], w.rearrange("(kt p) m -> p kt m", p=128)
            ).then_inc(s_w, 16)
            sync.dma_start(
                xs[:, :, :], data.rearrange("(kt p) n -> p kt n", p=128)
            ).then_inc(s_x, 16)
            sync.dma_start(wfs[:, :], wfft[:, :]).then_inc(s_cw, 16)
            sync.dma_start(tws[:, :], twid[:, :]).then_inc(s_ct, 16)
            # staging: mt 0-3 -> qdram, 4-7 -> kdram, 8-11 -> out_v
            for mt in range(12):
                sync.wait_ge(dve_sem, 8 * (mt + 1))
                dst_t = (qdram, kdram, out_v)[mt // 4]
                r = mt % 4
                dst = dst_t[128 * r : 128 * (r + 1), :].rearrange(
                    "p (ntc c) -> p ntc c", c=512
                )
                sync.dma_start(dst, evv[:, :, :]).then_inc(s_stage, 16)
            # R1 digit-relayout DMAs: ax[a, cc, b] = src[128*chunk+cc, 64a+b]
            for ct, (tname, chunk) in enumerate(ORDER):
                src_t = qdram if tname == "q" else kdram
                sync.wait_ge(s_stage, 16 * (4 if tname == "q" else 8))
                if ct > 0:
                    sync.wait_ge(pe_sem, _s2g(ct - 1))  # ax WAR vs prev S1
                src = src_t[128 * chunk : 128 * (chunk + 1), :].rearrange(
                    "cc (a b) -> a cc b", b=64
                )
                sync.dma_start(ax[0:64, :, :], src).then_inc(s_r1, 16)
            sync.wait_ge(cs_sem, 4)
            sync.dma_start(
                out_s.rearrange("pl h (v u) -> v pl h u", u=64),
                ssum[0:64, :, :, :],
            ).then_inc(s_out, 16)
            # Execution must not complete while output DMAs are in flight:
            # await out_v staging (DMAs 1-12) and the out_s DMA.
            sync.wait_ge(s_stage, 192)
            sync.wait_ge(s_out, 16)

        @block.tensor
        def _(tensor):
            tensor.wait_ge(s_w, 16)
            tensor.wait_ge(s_x, 16)
            g = 0
            for mt in range(12):
                for ntc in range(8):
                    if g >= NPS:
                        tensor.wait_ge(dve_sem, g - NPS + 1)
                    for kt in range(4):
                        mm = nc.tensor.matmul(
                            ps[:, g % NPS, :],
                            wt[:, kt, 128 * mt : 128 * (mt + 1)],
                            xs[:, kt, 512 * ntc : 512 * (ntc + 1)],
                            start=(kt == 0),
                            stop=(kt == 3),
                        )
                        if kt == 3:
                            mm.then_inc(pe_sem, 1)
                    g += 1
            tensor.wait_ge(s_cw, 16)
            for ct, (tname, chunk) in enumerate(ORDER):
                tensor.wait_ge(s_r1, 16 * (ct + 1))
                # S1: psum[b, (pl,u)] = sum_a ax[a, 2j+par, b] * [Wc|Wms][a,:]
                for par in range(2):
                    for j in range(64):
                        if g >= NPS:
                            tensor.wait_ge(dve_sem, g - NPS + 1)
                        nc.tensor.matmul(
                            ps[0:64, g % NPS, 0:128],
                            ax[0:64, 2 * j + par, :],
                            wfs[0:64, 0:128],
                            start=True,
                            stop=True,
                        ).then_inc(pe_sem, 1)
                        g += 1
                tensor.wait_ge(tw_sem, ct + 1)
                # S2: psum[v, 512] ; Z' planes in xs[0:64, 2*pl+par, :]
                for par in range(2):
                    for pch in range(8):
                        sl = slice(512 * pch, 512 * (pch + 1))
                        for pl in range(2):
                            if g >= NPS:
                                tensor.wait_ge(dve_sem, g - NPS + 1)
                            if pl == 0:  # Xr = Wc^T Zr + Ws^T Zi
                                nc.tensor.matmul(
                                    ps[0:64, g % NPS, :],
                                    wfs[0:64, 0:64],
                                    xs[0:64, par, sl],
                                    start=True,
                                    stop=False,
                                )
                                mm = nc.tensor.matmul(
                                    ps[0:64, g % NPS, :],
                                    wfs[0:64, 128:192],
                                    xs[0:64, 2 + par, sl],
                                    start=False,
                                    stop=True,
                                )
                            else:  # Xi = Wc^T Zi + Wms^T Zr
                                nc.tensor.matmul(
                                    ps[0:64, g % NPS, :],
                                    wfs[0:64, 0:64],
                                    xs[0:64, 2 + par, sl],
                                    start=True,
                                    stop=False,
                                )
                                mm = nc.tensor.matmul(
                                    ps[0:64, g % NPS, :],
                                    wfs[0:64, 64:128],
                                    xs[0:64, par, sl],
                                    start=False,
                                    stop=True,
                                )
                            mm.then_inc(pe_sem, 1)
                            g += 1

        @block.vector
        def _(vector):
            g = 0
            nc.vector.memset(ssum[0:64, :, :, :], 0.0).then_inc(ch_sem, 1)
            for mt in range(12):
                if mt >= 1:
                    vector.wait_ge(s_stage, 16 * mt)  # evv WAR vs staging DMA
                for ntc in range(8):
                    vector.wait_ge(pe_sem, g + 1)
                    nc.vector.tensor_copy(evv[:, ntc, :], ps[:, g % NPS, :]).then_inc(
                        dve_sem, 1
                    )
                    g += 1
            vector.wait_ge(s_ct, 16)
            ch = 1  # memset counted

            def chained(mk):
                # Dependent DVE->DVE chains need explicit ordering (engine
                # write visibility); serialize chained ops on ch_sem.
                nonlocal ch
                vector.wait_ge(ch_sem, ch)
                ins = mk()
                ins.then_inc(ch_sem, 1)
                ch += 1
                return ins

            for ct, (tname, chunk) in enumerate(ORDER):
                for par in range(2):
                    for j in range(64):
                        vector.wait_ge(pe_sem, g + 1)
                        nc.vector.tensor_copy(
                            gsb[0:64, par, j, :, :], ps[0:64, g % NPS, 0:128]
                        ).then_inc(dve_sem, 1)
                        g += 1
                # twiddle -> Z' into xs combos; WAR vs prev ct's S2 reads
                if ct > 0:
                    vector.wait_ge(pe_sem, _gend(ct - 1))
                if ct >= 2:
                    # tmp1 WAR vs previous chunk's final CS op
                    vector.wait_ge(cs_sem, ct // 2)
                vector.wait_ge(dve_sem, g)  # gsb evictions visible to DVE
                tc_b = _bcast_j(tws[0:64, 0:64])
                tms_b = _bcast_j(tws[0:64, 64:128])
                t1 = tmp1[0:64, :]
                last = None
                for par in range(2):
                    gr = gsb[0:64, par, :, 0, :]
                    gi = gsb[0:64, par, :, 1, :]
                    zr = xs[0:64, par, :]
                    zi = xs[0:64, 2 + par, :]
                    chained(lambda: nc.vector.scalar_tensor_tensor(zr, gr, 1.0, tc_b, MULT, MULT))
                    chained(lambda: nc.vector.scalar_tensor_tensor(t1, gi, 1.0, tms_b, MULT, MULT))
                    chained(lambda: nc.vector.scalar_tensor_tensor(zr, zr, 1.0, t1, MULT, SUB))
                    chained(lambda: nc.vector.scalar_tensor_tensor(t1, gr, 1.0, tms_b, MULT, MULT))
                    chained(lambda: nc.vector.scalar_tensor_tensor(zi, gi, 1.0, tc_b, MULT, MULT))
                    if par == 0:
                        chained(lambda: nc.vector.scalar_tensor_tensor(zi, zi, 1.0, t1, MULT, ADD))
                    else:
                        # final twiddle op: explicit chain wait, signals tw_sem
                        vector.wait_ge(ch_sem, ch)
                        nc.vector.scalar_tensor_tensor(
                            zi, zi, 1.0, t1, MULT, ADD
                        ).then_inc(tw_sem, 1)
                # S2 evictions
                xdst = xq if tname == "q" else xk
                for par in range(2):
                    for pch in range(8):
                        for pl in range(2):
                            vector.wait_ge(pe_sem, g + 1)
                            nc.vector.tensor_copy(
                                xdst[0:64, pl, par, 8 * pch : 8 * (pch + 1), :],
                                ps[0:64, g % NPS, :],
                            ).then_inc(dve_sem, 1)
                            g += 1
                if tname == "k":
                    # cross-spectrum for this chunk (heads 2c, 2c+1)
                    vector.wait_ge(dve_sem, g)  # X evictions visible to DVE
                    vector.wait_ge(tw_sem, ct + 1)  # final twiddle op done
                    if chunk > 0:
                        vector.wait_ge(cs_sem, chunk)  # ssum WAW vs prev chunk
                    for pl_out, (qa_i, kb_i, qc_i, kd_i, op) in enumerate(
                        [(0, 0, 1, 1, ADD), (1, 0, 0, 1, SUB)]
                    ):
                        for par in range(2):
                            is_final = pl_out == 1 and par == 1
                            qa = xq[0:64, qa_i, par, :, :]
                            kb = xk[0:64, kb_i, par, :, :]
                            qc = xq[0:64, qc_i, par, :, :]
                            kd = xk[0:64, kd_i, par, :, :]
                            t1f = tmp1[0:64, :]
                            t2f = tmp2[0:64, :]
                            chained(lambda: nc.vector.scalar_tensor_tensor(t1f, qa, 1.0, kb, MULT, MULT))
                            chained(lambda: nc.vector.scalar_tensor_tensor(t2f, qc, 1.0, kd, MULT, MULT))
                            chained(lambda: nc.vector.scalar_tensor_tensor(t1f, t1f, 1.0, t2f, MULT, op))
                            n = 16
                            while n >= 1:
                                dst = _tree_ap(tmp1, n, 0)
                                src = _tree_ap(tmp1, n, n * 64)
                                chained(lambda dst=dst, src=src: nc.vector.scalar_tensor_tensor(
                                    dst, dst, 1.0, src, MULT, ADD
                                ))
                                n //= 2
                            for hh in range(2):
                                red = tmp1[0:64, 2048 * hh : 2048 * hh + 64]
                                dst = ssum[0:64, pl_out, 2 * chunk + hh, :]
                                if is_final and hh == 1:
                                    # final CS op: explicit wait, signals cs_sem
                                    vector.wait_ge(ch_sem, ch)
                                    nc.vector.scalar_tensor_tensor(
                                        dst, dst, 1.0, red, MULT, ADD
                                    ).then_inc(cs_sem, 1)
                                else:
                                    chained(lambda red=red, dst=dst: nc.vector.scalar_tensor_tensor(
                                        dst, dst, 1.0, red, MULT, ADD
                                    ))

    return nc


def _get_graphs():
    if not _GRAPHS:
        _GRAPHS["proj"] = qkcorr_graph()
        _GRAPHS["outproj"] = _mm_graph(L, DM)
    return _GRAPHS


LAST_EXEC_NS = [None, None]


def _bf16(a):
    return np.ascontiguousarray(a, dtype=NPBF16)


def kernel(x, Wq, bq, Wk, bk, Wv, bv, Wo, bo):
    x = np.asarray(x, np.float32)
    Wq, bq = np.asarray(Wq, np.float32), np.asarray(bq, np.float32)
    Wk, bk = np.asarray(Wk, np.float32), np.asarray(bk, np.float32)
    Wv, bv = np.asarray(Wv, np.float32), np.asarray(bv, np.float32)
    Wo, bo = np.asarray(Wo, np.float32), np.asarray(bo, np.float32)
    g = _get_graphs()

    # ---- device graph A: QKV projections + on-device cross-spectrum ----
    # (b-sharding on 4 cores dedupes the x upload; Q,K never cross the
    # tunnel - only V and the tiny 8-head spectrum come back.)
    w_all = _bf16(np.concatenate([Wq.T, Wk.T, Wv.T], axis=1))  # (DM, 1536)
    wfft_np, twid_np = fft_consts()
    in_maps = [
        {"data": _bf16(x[b].T), "w": w_all, "wfft": wfft_np, "twid": twid_np}
        for b in range(B)
    ]
    resA = run_bass_kernel_spmd(g["proj"], in_maps, core_ids=list(range(B)))
    LAST_EXEC_NS[0] = resA.exec_time_ns

    # V (channel-major) with bias; spectrum -> corr via host iFFT.
    # Bias on Q,K only shifts corr by a per-head constant (DC bin), which
    # top-k order and softmax are both invariant to, so it is skipped.
    VT = np.empty((B, DM, L), np.float32)
    corr_mean = np.empty((B, H, L), np.float64)
    for b in range(B):
        VT[b] = np.asarray(resA.results[b]["out_v"]).astype(np.float32) + bv[:, None]
        s_ri = np.asarray(resA.results[b]["out_s"])  # (2, 8, L) f32
        S = s_ri[0].astype(np.float64) + 1j * s_ri[1].astype(np.float64)
        corr_mean[b] = np.fft.ifft(S, axis=-1).real / DK

    k = min(int(2 * math.log(L)), L)  # 16
    order = np.argsort(-corr_mean, axis=-1, kind="stable")
    delays = order[..., :k]  # (B, H, k)
    wvals = np.take_along_axis(corr_mean, delays, axis=-1)
    wvals = wvals - wvals.max(axis=-1, keepdims=True)
    wexp = np.exp(wvals)
    wsm = (wexp / wexp.sum(axis=-1, keepdims=True)).astype(np.float32)

    # ctxT[b, c, t] = sum_j w_j * VT[b, c, (t - delay_j) % L]
    ctxT = np.zeros((B, DM, L), np.float32)
    for b in range(B):
        for h in range(H):
            csl = slice(64 * h, 64 * (h + 1))
            acc = ctxT[b, csl]
            for j in range(k):
                d = int(delays[b, h, j])
                wj = wsm[b, h, j]
                rolled = np.concatenate(
                    [VT[b, csl, L - d :], VT[b, csl, : L - d]], axis=1
                )
                acc += wj * rolled
    # ---- device graph B: output projection, one core per batch ----
    woT = _bf16(Wo.T)
    in_maps_b = [{"data": _bf16(ctxT[b]), "w": woT} for b in range(B)]
    resB = run_bass_kernel_spmd(g["outproj"], in_maps_b, core_ids=list(range(B)))
    LAST_EXEC_NS[1] = resB.exec_time_ns

    out = np.empty((B, L, DM), np.float32)
    for b in range(B):
        ob = np.asarray(resB.results[b]["out"]).astype(np.float32)  # (512, L)
        out[b] = ob.T
    out += bo.astype(np.float32)
    return out
